# revision 56
# baseline (speedup 1.0000x reference)
"""AttentionSubsample kernel for 8 trn2 NeuronCores.

Sharding: head-parallel (8 heads -> 8 cores). Each core runs its head through
kv/q projection, attention and hardswish, then computes a PARTIAL output
projection (512 out channels x its 32 v-channels) which is summed across
cores with a per-q-chunk ReduceScatter; core i keeps output rows 64i:64i+64,
so the final BatchNorm is purely local.

Schedule notes:
- x is DMA'd in 4 token chunks; kv projection, PSUM-side bn_stats and drains
  trail each chunk so the kv-stat barrier lands right after the last DMA.
- BN folding: the k-side BatchNorm shift cancels inside the softmax (it is
  constant per query), so QK runs on RAW k with q~ = (s_k*s_q) q_raw +
  s_k*t_q.  The v-side BatchNorm is folded into the post-attention drain:
  out = (av * s_v) * (1/denom) + t_v via scalar_tensor_tensor.
- Attention phase keeps the Activation engine exp-only; all exp(bias)
  multiplies run on DVE (gpsimd multiply efficiency is only 0.42).
- Softmax denominator from a ones-column in the AV stationary (33rd col).
- q chunks are [512, 512, 320]: the final (tail) ReduceScatter is small and
  starts as early as possible.
- The post-collective ops are issued under tile_wait_until so the scheduler
  cannot hoist them into the attention phase (head-of-line queue blocking).
"""

import numpy as np
import ml_dtypes

import concourse.bass as bass
import concourse.mybir as mybir
import concourse.tile as tile
from concourse import bacc
from contextlib import ExitStack
from concourse.bass_utils import run_bass_kernel_spmd

BF16 = mybir.dt.bfloat16
F32 = mybir.dt.float32
bf16 = ml_dtypes.bfloat16
F8 = mybir.dt.float8e4
f8 = ml_dtypes.float8_e4m3

B = 2
ROW, COL = 63, 84
ROW_, COL_ = 32, 42
N = ROW * COL            # 5292 kv tokens
NQ = ROW_ * COL_         # 1344 q tokens
NPAD = 5376              # 42*128 padded kv tokens
KT = NPAD // 128         # 42 k-tiles
QCS = [512, 512, 320]    # q chunks (tail chunk small -> short last collective)
QOF = [0, 512, 1024]
NQC = len(QCS)
CIN = 256
H = 8
KD = 16
DV = 32
HKV = KD + DV            # 48 per-head kv channels
KVP = 64                 # padded kv rows: k at 0:16, v at 32:64
OC = 64                  # per-core slice of the 512 output channels
GRP = 2                  # k-tiles per exp group
NGRP = KT // GRP         # 21
EPS = 1e-5
SCALE = KD ** -0.5
NCORES = 8
XCS = [1344, 1344, 1344, 896, 448]   # x DMA chunks (small last chunk ->
XOF = [0, 1344, 2688, 4032, 4928]    # short post-DMA stats chain)

LAST_EXEC_NS = None
_prog_cache = {}


def _build_program(debug=False):
    nc = bacc.Bacc(num_devices=NCORES)

    xT = nc.dram_tensor("xT", [B, 2, 128, NPAD], BF16, kind="ExternalInput")
    xsT = nc.dram_tensor("xsT", [B, 2, 128, NQ], BF16, kind="ExternalInput")
    wkqT = nc.dram_tensor("wkqT", [2, 128, KVP + KD], BF16,
                          kind="ExternalInput")
    wpT = nc.dram_tensor("wpT", [DV, 4, 128], BF16, kind="ExternalInput")
    gbT = nc.dram_tensor("gbT", [KVP, 6], F32, kind="ExternalInput")
    idT = nc.dram_tensor("idT", [KVP, DV], BF16, kind="ExternalInput")
    ebT = nc.dram_tensor("ebT", [NGRP, 128, GRP, NQ], BF16,
                         kind="ExternalInput")
    yT = nc.dram_tensor("yT", [OC, B * NQ], F32, kind="ExternalOutput")
    if debug:
        dbg = {
            "dbg_ykv": nc.dram_tensor("dbg_ykv", [KVP, B, NPAD], BF16,
                                      kind="ExternalOutput"),
            "dbg_qT": nc.dram_tensor("dbg_qT", [KD, B, NQ], BF16,
                                     kind="ExternalOutput"),
            "dbg_vaug": nc.dram_tensor("dbg_vaug", [128, B, KT, DV + 1], BF16,
                                       kind="ExternalOutput"),
            "dbg_hsT": nc.dram_tensor("dbg_hsT", [DV, B, NQ], BF16,
                                      kind="ExternalOutput"),
            "dbg_mvkv": nc.dram_tensor("dbg_mvkv", [KVP, 2], F32,
                                       kind="ExternalOutput"),
            "dbg_yfin": nc.dram_tensor("dbg_yfin", [OC, B, NQ], BF16,
                                       kind="ExternalOutput"),
        }

    with ExitStack() as ctx:
        tc = ctx.enter_context(tile.TileContext(nc))
        const = ctx.enter_context(tc.tile_pool(name="const", bufs=1))
        big = ctx.enter_context(tc.tile_pool(name="big", bufs=1))
        spool = ctx.enter_context(tc.tile_pool(name="spool", bufs=8))
        ebpool = ctx.enter_context(tc.tile_pool(name="ebpool", bufs=10))
        small = ctx.enter_context(tc.tile_pool(name="small", bufs=4))
        drain = ctx.enter_context(tc.tile_pool(name="drain", bufs=3))
        psA = ctx.enter_context(tc.tile_pool(name="psA", bufs=2, space="PSUM"))
        psB = ctx.enter_context(tc.tile_pool(name="psB", bufs=2, space="PSUM"))
        psC = ctx.enter_context(tc.tile_pool(name="psC", bufs=2, space="PSUM"))
        dram = ctx.enter_context(tc.tile_pool(name="dram", bufs=4, space="DRAM"))

        mult = mybir.AluOpType.mult
        add = mybir.AluOpType.add
        amin = mybir.AluOpType.min
        amax = mybir.AluOpType.max
        Act = mybir.ActivationFunctionType

        wkq_sb = const.tile([128, 2, KVP + KD], BF16, tag="wkq")
        wp_sb = const.tile([DV, 4, 128], BF16, tag="wp")
        gb_sb = const.tile([KVP, 6], F32, tag="gb")
        id_sb = const.tile([KVP, DV], BF16, tag="id")
        eps_t = const.tile([128, 1], F32, tag="eps")
        nc.vector.memset(eps_t, EPS)
        ones1_t = const.tile([1, DV], F32, tag="ones1")
        nc.vector.memset(ones1_t, 1.0)

        # v_aug gets its ones column once; the raw-v transposes fill 0:DV.
        v_aug = big.tile([128, B, KT, DV + 1], BF16, tag="vaug")
        nc.gpsimd.memset(v_aug[:, :, :, DV:DV + 1], 1.0)

        xt_sb = big.tile([128, B, 2, NPAD], BF16, tag="xt")
        xs_sb = big.tile([128, B, 2, NQ], BF16, tag="xs")
        y_kv = big.tile([KVP, B, NPAD], BF16, tag="ykv")
        y_q = big.tile([KD, B, NQ], BF16, tag="yq")
        st_kv = small.tile([KVP, 24, 6], F32, tag="st_kv")
        st_q = small.tile([KD, 6, 6], F32, tag="st_q")

        # ------------- pipelined x DMA + kv projection + stats -------------
        def kv_chunk(ch):
            X0, XW = XOF[ch], XCS[ch]
            for b in range(B):
                for c in range(2):
                    nc.sync.dma_start(
                        out=xt_sb[:, b, c, bass.ds(X0, XW)],
                        in_=xT[b, c, :, bass.ds(X0, XW)])
            if ch == 0:
                # weights land between chunk0 and chunk1 transfers
                for c in range(2):
                    nc.sync.dma_start(out=wkq_sb[:, c, :], in_=wkqT[c])
                nc.sync.dma_start(out=wp_sb, in_=wpT[:, :, :])
                nc.sync.dma_start(out=gb_sb, in_=gbT[:, :])
                nc.sync.dma_start(out=id_sb, in_=idT[:, :])
            for b in range(B):
                for u in range(XW // 448):
                    t = (X0 // 448) + u
                    ps = psB.tile([KVP, 448], F32, tag="ps_av")
                    for c in range(2):
                        nc.tensor.matmul(ps, wkq_sb[:, c, 0:KVP],
                                         xt_sb[:, b, c, bass.ds(t * 448, 448)],
                                         start=(c == 0), stop=(c == 1))
                    if t < 9 and (t + b) % 2 == 0:
                        nc.vector.tensor_copy(
                            y_kv[:, b, bass.ds(t * 448, 448)], ps)
                    else:
                        nc.scalar.copy(out=y_kv[:, b, bass.ds(t * 448, 448)],
                                       in_=ps)
                    # stats off the drained y_kv so the PSUM pipeline is
                    # drain-rate-limited (pads excluded via window width)
                    w = min(448, N - t * 448)
                    nc.vector.bn_stats(out=st_kv[:, t * B + b, :],
                                       in_=y_kv[:, b, bass.ds(t * 448, w)])

        kv_chunk(0)
        kv_chunk(1)
        # xs lands while kv chunk 2 streams
        for b in range(B):
            for c in range(2):
                nc.sync.dma_start(out=xs_sb[:, b, c, :], in_=xsT[b, c])
        kv_chunk(2)
        # q projection slots in while kv chunks 3/4 stream
        for b in range(B):
            for t in range(3):
                ps = psB.tile([KD, 448], F32, tag="ps_av")
                for c in range(2):
                    nc.tensor.matmul(ps, wkq_sb[:, c, KVP:KVP + KD],
                                     xs_sb[:, b, c, bass.ds(t * 448, 448)],
                                     start=(c == 0), stop=(c == 1))
                nc.scalar.copy(out=y_q[:, b, bass.ds(t * 448, 448)], in_=ps)
            for t in range(3):
                nc.vector.bn_stats(out=st_q[:, b * 3 + t, :],
                                   in_=y_q[:, b, bass.ds(t * 448, 448)])
        kv_chunk(3)
        kv_chunk(4)

        # ------------- batch-norm scale/shift + q~ -------------
        def bn_scale_shift(mv, g_ap, b_ap, P, name):
            s = small.tile([P, 1], F32, tag=f"s_{name}")
            t = small.tile([P, 1], F32, tag=f"t_{name}")
            nc.scalar.activation(out=s, in_=mv[:, 1:2], func=Act.Sqrt,
                                 bias=eps_t[0:P])
            nc.vector.reciprocal(out=s, in_=s)
            nc.vector.tensor_mul(s, s, g_ap)
            nc.vector.tensor_mul(t, mv[:, 0:1], s)
            nc.vector.tensor_scalar(out=t, in0=t, scalar1=-1.0, scalar2=None,
                                    op0=mult)
            nc.vector.tensor_add(t, t, b_ap)
            return s, t

        mv_kv = small.tile([KVP, 2], F32, tag="mv_kv")
        nc.vector.bn_aggr(out=mv_kv, in_=st_kv)
        s_kv, t_kv = bn_scale_shift(mv_kv, gb_sb[:, 0:1], gb_sb[:, 1:2],
                                    KVP, "kv")
        mv_q = small.tile([KD, 2], F32, tag="mv_q")
        nc.vector.bn_aggr(out=mv_q, in_=st_q)
        s_q, t_q = bn_scale_shift(mv_q, gb_sb[0:KD, 2:3], gb_sb[0:KD, 3:4],
                                  KD, "q")

        # q~ = (s_k*s_q) . q_raw + s_k*t_q
        a_q = small.tile([KD, 1], F32, tag="a_q")
        b_q = small.tile([KD, 1], F32, tag="b_q")
        nc.vector.tensor_mul(a_q, s_kv[0:KD], s_q)
        nc.vector.tensor_mul(b_q, s_kv[0:KD], t_q)
        qT = big.tile([KD, B, NQ], BF16, tag="qT")
        for b in range(B):
            nc.vector.tensor_scalar(out=qT[:, b, :], in0=y_q[:, b, :],
                                    scalar1=a_q, scalar2=b_q,
                                    op0=mult, op1=add)
        # v-side scale/shift moved to base partition 0 for the drain ops
        s_v0 = small.tile([DV, 1], F32, tag="s_v0")
        t_v0 = small.tile([DV, 1], F32, tag="t_v0")
        nc.gpsimd.dma_start(out=s_v0, in_=s_kv[32:KVP])
        nc.gpsimd.dma_start(out=t_v0, in_=t_kv[32:KVP])
        c3_v = small.tile([DV, 1], F32, tag="c3v")
        nc.vector.tensor_scalar(out=c3_v, in0=t_v0, scalar1=3.0,
                                scalar2=None, op0=add)
        # raw v -> token-major via PE transposes (the DMA transpose engine
        # serializes against later DMAs in the tile scheduler, so avoid it)
        for b in range(B):
            for t0 in range(0, KT, 32):
                nt = min(32, KT - t0)
                pc_t = psC.tile([128, 1024], BF16, tag="pc")
                for i in range(nt):
                    nc.tensor.transpose(
                        pc_t[:, bass.ts(i, DV)],
                        y_kv[32:KVP, b, bass.ds((t0 + i) * 128, 128)],
                        id_sb[32:KVP, :])
                nc.vector.tensor_copy(v_aug[:, b, t0:t0 + nt, 0:DV],
                                      pc_t[:, 0:nt * DV])

        # ------------- attention + partial projection -------------
        hsT = big.tile([DV, B, NQ], BF16, tag="hsT")
        y_fin = big.tile([OC, B, NQ], BF16, tag="y_fin")
        y_out = big.tile([OC, B, NQ], F32, tag="y_out")
        y_bncs = []
        y_sls = []
        for i in range(NQC):
            ybnc_t = dram.tile([4, 128, B * QCS[i]], BF16, tag=f"bnc{i}")
            y_bncs.append(ybnc_t)
            ysl_t = dram.tile([OC, B * QCS[i]], BF16, tag=f"ysl{i}")
            y_sls.append(ysl_t)

        ypars = []
        for i in range(NQC):
            ypar_t = big.tile([128, 4, B, QCS[i]], BF16, tag=f"ypar{i}")
            ypars.append(ypar_t)

        def emit_proj_piece(qc, o, tail=False):
            # one <=512-wide partial-projection matmul per (b, out-block)
            W = QCS[qc]
            q0 = QOF[qc]
            for b in range(B):
                pc = psC.tile([128, 512], F32, tag="pc")
                nc.tensor.matmul(pc[:, 0:W], wp_sb[:, o, :],
                                 hsT[:, b, bass.ds(q0, W)],
                                 start=True, stop=True)
                dst = ypars[qc][:, o, b, 0:W]
                if tail and b == 0:
                    nc.scalar.copy(out=dst, in_=pc[:, 0:W])
                else:
                    nc.vector.tensor_copy(dst, pc[:, 0:W])

        def emit_rs(qc, tail=False):
            dma = nc.scalar.dma_start if tail else nc.gpsimd.dma_start
            dma(out=y_bncs[qc].rearrange("o p (b q) -> p o b q", b=B),
                in_=ypars[qc])
            nc.gpsimd.collective_compute(
                "ReduceScatter", add,
                replica_groups=[list(range(NCORES))],
                ins=[y_bncs[qc].opt()],
                outs=[y_sls[qc].opt()])

        def attn_group(g, b, av_t, eb, W, q0):
            qk = psA.tile([128, GRP, 512], F32, tag="qk")
            for i in range(GRP):
                j = g * GRP + i
                nc.tensor.matmul(qk[:, i, 0:W],
                                 y_kv[0:KD, b, bass.ts(j, 128)],
                                 qT[:, b, bass.ds(q0, W)],
                                 start=True, stop=True)
            sp = spool.tile([128, GRP, 512], BF16, tag="sp")
            nc.scalar.activation(out=sp[:, :, 0:W], in_=qk[:, :, 0:W],
                                 func=Act.Exp, scale=SCALE)
            nc.vector.tensor_mul(sp[:, :, 0:W], sp[:, :, 0:W], eb[:, :, 0:W])
            for i in range(GRP):
                j = g * GRP + i
                nc.tensor.matmul(av_t[:, 0:W], v_aug[:, b, j, :],
                                 sp[:, i, 0:W],
                                 start=(j == 0), stop=(j == KT - 1),
                                 skip_group_check=True)

        def drain_chain(b, av_t, W, q0, alt):
            # out = (av*s_v)/denom + t_v, then hardswish.  alt=True parks the
            # accumulator via ACT (tail: both batches drain in parallel).
            av = drain.tile([DV + 1, 512], F32, tag="av_sb")
            if alt:
                nc.scalar.copy(out=av[:, 0:W], in_=av_t[:, 0:W])
            else:
                nc.vector.tensor_copy(av[:, 0:W], av_t[:, 0:W])
            rec = drain.tile([1, 512], F32, tag="rec")
            nc.vector.reciprocal(out=rec[:, 0:W], in_=av[DV:DV + 1, 0:W])
            pc_r = psC.tile([128, 512], F32, tag="pc")
            recb = pc_r[0:DV, :]
            nc.tensor.matmul(recb[:, 0:W], ones1_t, rec[:, 0:W],
                             start=True, stop=True)
            xo = drain.tile([DV, 512], F32, tag="xo")
            nc.vector.scalar_tensor_tensor(
                out=xo[:, 0:W], in0=av[0:DV, 0:W], scalar=s_v0,
                in1=recb[:, 0:W], op0=mult, op1=mult)
            r3 = drain.tile([DV, 512], F32, tag="r3")
            nc.vector.tensor_scalar(out=r3[:, 0:W], in0=xo[:, 0:W],
                                    scalar1=c3_v, scalar2=0.0,
                                    op0=add, op1=amax)
            nc.vector.tensor_scalar(out=r3[:, 0:W], in0=r3[:, 0:W],
                                    scalar1=6.0, scalar2=1.0 / 6.0,
                                    op0=amin, op1=mult)
            nc.vector.scalar_tensor_tensor(
                out=hsT[:, b, bass.ds(q0, W)], in0=xo[:, 0:W],
                scalar=t_v0, in1=r3[:, 0:W], op0=add, op1=mult)

        for qc in range(NQC - 1):
            W = QCS[qc]
            q0 = QOF[qc]
            avs = []
            for _b in range(B):
                av_t = psB.tile([DV + 1, 512], F32, tag="ps_av")
                avs.append(av_t)
            for g in range(NGRP):
                eb = ebpool.tile([128, GRP, 512], BF16, tag="eb")
                nc.sync.dma_start(out=eb[:, :, 0:W],
                                  in_=ebT[g, :, :, bass.ds(q0, W)])
                if qc > 0 and 5 <= g < 13 and g % 2 == 1:
                    emit_proj_piece(qc - 1, (g - 5) // 2)
                if qc > 0 and g == 13:
                    emit_rs(qc - 1)
                for b in range(B):
                    attn_group(g, b, avs[b], eb, W, q0)
            for b in range(B):
                drain_chain(b, avs[b], W, q0, False)
        # ---- tail chunk, b-outer: b0's drain/proj/bounce hide under b1 ----
        qc = NQC - 1
        W = QCS[qc]
        q0 = QOF[qc]
        for b in range(B):
            av_t = psB.tile([DV + 1, 512], F32, tag="ps_av")
            for g in range(NGRP):
                eb = ebpool.tile([128, GRP, 512], BF16, tag="eb")
                nc.sync.dma_start(out=eb[:, :, 0:W],
                                  in_=ebT[g, :, :, bass.ds(q0, W)])
                if b == 0 and 5 <= g < 13 and g % 2 == 1:
                    emit_proj_piece(qc - 1, (g - 5) // 2)
                if b == 0 and g == 13:
                    emit_rs(qc - 1)
                attn_group(g, b, av_t, eb, W, q0)
            drain_chain(b, av_t, W, q0, alt=(b == 1))
            for o in range(4):
                pc = psC.tile([128, 512], F32, tag="pc")
                nc.tensor.matmul(pc[:, 0:W], wp_sb[:, o, :],
                                 hsT[:, b, bass.ds(q0, W)],
                                 start=True, stop=True)
                dst = ypars[qc][:, o, b, 0:W]
                if b == 0:
                    nc.vector.tensor_copy(dst, pc[:, 0:W])
                else:
                    nc.scalar.copy(out=dst, in_=pc[:, 0:W])
            half_dma = nc.gpsimd.dma_start
            half_dma(
                out=y_bncs[qc][:, :, bass.ds(b * W, W)].rearrange(
                    "o p q -> p o q"),
                in_=ypars[qc][:, :, b, :])
        nc.gpsimd.collective_compute(
            "ReduceScatter", add,
            replica_groups=[list(range(NCORES))],
            ins=[y_bncs[qc].opt()],
            outs=[y_sls[qc].opt()])
        # preload the Sqrt activation table while the collective runs
        sq_warm = small.tile([1, 1], F32, tag="sq_warm")
        nc.scalar.activation(out=sq_warm, in_=eps_t[0:1, :], func=Act.Sqrt,
                             bias=eps_t[0:1])

        if debug:
            nc.sync.dma_start(out=dbg["dbg_ykv"][:, :, :], in_=y_kv)
            nc.sync.dma_start(out=dbg["dbg_qT"][:, :, :], in_=qT)
            nc.sync.dma_start(out=dbg["dbg_vaug"][:, :, :, :], in_=v_aug)
            nc.sync.dma_start(out=dbg["dbg_hsT"][:, :, :], in_=hsT)
            nc.sync.dma_start(out=dbg["dbg_mvkv"][:, :], in_=mv_kv)

        # ------------- gather slices + final BN (kept off queue heads) -----
        ctx.enter_context(tc.tile_wait_until(0.25))
        st_p = small.tile([OC, B * NQC, 6], F32, tag="st_p")
        for qc in range(NQC):
            W = QCS[qc]
            for b in range(B):
                nc.sync.dma_start(
                    out=y_fin[:, b, bass.ds(QOF[qc], W)],
                    in_=y_sls[qc][:, bass.ds(b * W, W)])
                nc.vector.bn_stats(out=st_p[:, qc * B + b, :],
                                   in_=y_fin[:, b, bass.ds(QOF[qc], W)])
        mv_p = small.tile([OC, 2], F32, tag="mv_p")
        nc.vector.bn_aggr(out=mv_p, in_=st_p)
        s_p, t_p = bn_scale_shift(mv_p, gb_sb[:, 4:5], gb_sb[:, 5:6],
                                  OC, "p")
        nc.vector.tensor_scalar(out=y_out[:, 0, :], in0=y_fin[:, 0, :],
                                scalar1=s_p, scalar2=t_p, op0=mult, op1=add)
        nc.vector.tensor_scalar(out=y_out[:, 1, :], in0=y_fin[:, 1, :],
                                scalar1=s_p, scalar2=t_p, op0=mult, op1=add)
        if debug:
            nc.sync.dma_start(out=dbg["dbg_yfin"][:, :, :], in_=y_fin)
        nc.sync.dma_start(out=yT[:, 0:NQ], in_=y_out[:, 0, :])
        nc.sync.dma_start(out=yT[:, NQ:2 * NQ], in_=y_out[:, 1, :])

    nc.finalize()
    return nc


def _prep_inputs(x, kv_w, kv_g, kv_b, q_w, q_g, q_b, proj_w, proj_g, proj_b,
                 bias_table, bias_idxs):
    """Host-side sharding/layout prep. Returns list of 8 per-core input maps."""
    x = np.asarray(x, np.float32)
    xt = np.zeros((B, 2, 128, NPAD), np.float32)
    xTt = x.transpose(0, 2, 1)  # (B, 256, N)
    xt[:, :, :, :N] = xTt.reshape(B, 2, 128, N)
    xt = xt.astype(bf16)

    # exp(bias) tables per head, padded-k zeroed, laid out (NGRP, 128, GRP, NQ)
    rank2 = np.asarray(bias_idxs)[0].reshape(ROW, COL)  # (dr, dc) -> id
    table2 = np.asarray(bias_table, np.float32)[:, rank2]  # (H, 63, 84)
    eb2 = np.exp(table2)
    kk = np.arange(N)
    qq = np.arange(NQ)
    DRm = np.abs(kk[:, None] // COL - 2 * (qq[None, :] // COL_))
    DCm = np.abs(kk[:, None] % COL - 2 * (qq[None, :] % COL_))

    xs = x.reshape(B, ROW, COL, CIN)[:, ::2, ::2].reshape(B, NQ, CIN)
    xst = xs.transpose(0, 2, 1).reshape(B, 2, 128, NQ).astype(bf16)
    in_maps = []
    for h in range(H):
        ebf = np.zeros((NPAD, NQ), np.float32)
        ebf[:N] = eb2[h][DRm, DCm]
        ebl = np.ascontiguousarray(
            ebf.reshape(NGRP, GRP, 128, NQ).transpose(0, 2, 1, 3)
        ).astype(bf16)
        sl = slice(h * HKV, (h + 1) * HKV)
        slq = slice(h * KD, (h + 1) * KD)
        slo = slice(h * OC, (h + 1) * OC)
        wkv_pad = np.zeros((KVP, CIN), np.float32)
        wkv_pad[0:KD] = np.asarray(kv_w, np.float32)[sl][0:KD]
        wkv_pad[32:KVP] = np.asarray(kv_w, np.float32)[sl][KD:HKV]
        # combined kv+q weights: (2, 128, KVP+KD)
        wkq = np.zeros((CIN, KVP + KD), np.float32)
        wkq[:, 0:KVP] = wkv_pad.T
        wkq[:, KVP:] = np.asarray(q_w, np.float32)[slq].T
        # packed gains/biases: kv at [:,0:2], q at rows 0:16 [:,2:4],
        # proj at [:,4:6]
        gb = np.zeros((KVP, 6), np.float32)
        gb[:, 0] = 1.0
        gb[0:KD, 0] = np.asarray(kv_g, np.float32)[sl][0:KD]
        gb[0:KD, 1] = np.asarray(kv_b, np.float32)[sl][0:KD]
        gb[32:KVP, 0] = np.asarray(kv_g, np.float32)[sl][KD:HKV]
        gb[32:KVP, 1] = np.asarray(kv_b, np.float32)[sl][KD:HKV]
        gb[0:KD, 2] = np.asarray(q_g, np.float32)[slq]
        gb[0:KD, 3] = np.asarray(q_b, np.float32)[slq]
        gb[:, 4] = np.asarray(proj_g, np.float32)[slo]
        gb[:, 5] = np.asarray(proj_b, np.float32)[slo]
        # W_p columns for this head's 32 v-channels, as 4 stationary tiles
        wp_h = np.asarray(proj_w, np.float32)[:, h * DV:(h + 1) * DV]
        wp_t = np.ascontiguousarray(wp_h.T.reshape(DV, 4, 128)).astype(bf16)
        in_maps.append({
            "xT": xt,
            "xsT": xst,
            "idT": np.concatenate([np.zeros((DV, DV), np.float32), np.eye(DV, dtype=np.float32)]).astype(bf16),
            "wkqT": np.ascontiguousarray(
                wkq.reshape(2, 128, KVP + KD)).astype(bf16),
            "wpT": wp_t,
            "gbT": np.ascontiguousarray(gb),
            "ebT": ebl,
        })
    return in_maps


def kernel(x, kv_w, kv_g, kv_b, q_w, q_g, q_b, proj_w, proj_g, proj_b,
           bias_table, bias_idxs, _trace=False):
    global LAST_EXEC_NS
    if "nc" not in _prog_cache:
        _prog_cache["nc"] = _build_program()
    nc = _prog_cache["nc"]
    in_maps = _prep_inputs(x, kv_w, kv_g, kv_b, q_w, q_g, q_b,
                           proj_w, proj_g, proj_b, bias_table, bias_idxs)
    res = run_bass_kernel_spmd(nc, in_maps, core_ids=list(range(NCORES)),
                               trace=_trace)
    LAST_EXEC_NS = res.exec_time_ns
    yts = [np.asarray(r["yT"]) for r in res.results]  # each (OC, B*NQ)
    y = np.concatenate(yts, axis=0)                   # (512, B*NQ)
    return np.ascontiguousarray(
        y.T.reshape(B, NQ, H * OC).astype(np.float32))


# revision 57
# speedup vs baseline: 1.0081x; 1.0081x over previous
"""AttentionSubsample kernel for 8 trn2 NeuronCores.

Sharding: head-parallel (8 heads -> 8 cores). Each core runs its head through
kv/q projection, attention and hardswish, then computes a PARTIAL output
projection (512 out channels x its 32 v-channels) which is summed across
cores with a per-q-chunk ReduceScatter; core i keeps output rows 64i:64i+64,
so the final BatchNorm is purely local.

Schedule notes:
- x is DMA'd in 4 token chunks; kv projection, PSUM-side bn_stats and drains
  trail each chunk so the kv-stat barrier lands right after the last DMA.
- BN folding: the k-side BatchNorm shift cancels inside the softmax (it is
  constant per query), so QK runs on RAW k with q~ = (s_k*s_q) q_raw +
  s_k*t_q.  The v-side BatchNorm is folded into the post-attention drain:
  out = (av * s_v) * (1/denom) + t_v via scalar_tensor_tensor.
- Attention phase keeps the Activation engine exp-only; all exp(bias)
  multiplies run on DVE (gpsimd multiply efficiency is only 0.42).
- Softmax denominator from a ones-column in the AV stationary (33rd col).
- q chunks are [512, 512, 320]: the final (tail) ReduceScatter is small and
  starts as early as possible.
- The post-collective ops are issued under tile_wait_until so the scheduler
  cannot hoist them into the attention phase (head-of-line queue blocking).
"""

import numpy as np
import ml_dtypes

import concourse.bass as bass
import concourse.mybir as mybir
import concourse.tile as tile
from concourse import bacc
from contextlib import ExitStack
from concourse.bass_utils import run_bass_kernel_spmd

BF16 = mybir.dt.bfloat16
F32 = mybir.dt.float32
bf16 = ml_dtypes.bfloat16
F8 = mybir.dt.float8e4
f8 = ml_dtypes.float8_e4m3

B = 2
ROW, COL = 63, 84
ROW_, COL_ = 32, 42
N = ROW * COL            # 5292 kv tokens
NQ = ROW_ * COL_         # 1344 q tokens
NPAD = 5376              # 42*128 padded kv tokens
KT = NPAD // 128         # 42 k-tiles
QCS = [512, 512, 320]    # q chunks (tail chunk small -> short last collective)
QOF = [0, 512, 1024]
NQC = len(QCS)
CIN = 256
H = 8
KD = 16
DV = 32
HKV = KD + DV            # 48 per-head kv channels
KVP = 64                 # padded kv rows: k at 0:16, v at 32:64
OC = 64                  # per-core slice of the 512 output channels
GRP = 2                  # k-tiles per exp group
NGRP = KT // GRP         # 21
EPS = 1e-5
SCALE = KD ** -0.5
NCORES = 8
XCS = [1344, 1344, 1344, 896, 448]   # x DMA chunks (small last chunk ->
XOF = [0, 1344, 2688, 4032, 4928]    # short post-DMA stats chain)


LAST_EXEC_NS = None
_prog_cache = {}


def _build_program(debug=False):
    nc = bacc.Bacc(num_devices=NCORES)

    xT = nc.dram_tensor("xT", [B, 2, 128, NPAD], BF16, kind="ExternalInput")
    xsT = nc.dram_tensor("xsT", [B, 2, 128, NQ], BF16, kind="ExternalInput")
    wkqT = nc.dram_tensor("wkqT", [2, 128, KVP + KD], BF16,
                          kind="ExternalInput")
    wpT = nc.dram_tensor("wpT", [DV, 4, 128], BF16, kind="ExternalInput")
    gbT = nc.dram_tensor("gbT", [KVP, 6], F32, kind="ExternalInput")
    idT = nc.dram_tensor("idT", [KVP, DV], BF16, kind="ExternalInput")
    ebT = nc.dram_tensor("ebT", [NGRP, 128, GRP, NQ], BF16,
                         kind="ExternalInput")
    yT = nc.dram_tensor("yT", [OC, B * NQ], F32, kind="ExternalOutput")
    if debug:
        dbg = {
            "dbg_ykv": nc.dram_tensor("dbg_ykv", [KVP, B, NPAD], BF16,
                                      kind="ExternalOutput"),
            "dbg_qT": nc.dram_tensor("dbg_qT", [KD, B, NQ], BF16,
                                     kind="ExternalOutput"),
            "dbg_vaug": nc.dram_tensor("dbg_vaug", [128, B, KT, DV + 1], BF16,
                                       kind="ExternalOutput"),
            "dbg_hsT": nc.dram_tensor("dbg_hsT", [DV, B, NQ], BF16,
                                      kind="ExternalOutput"),
            "dbg_mvkv": nc.dram_tensor("dbg_mvkv", [KVP, 2], F32,
                                       kind="ExternalOutput"),
            "dbg_yfin": nc.dram_tensor("dbg_yfin", [OC, B, NQ], BF16,
                                       kind="ExternalOutput"),
        }

    with ExitStack() as ctx:
        tc = ctx.enter_context(tile.TileContext(nc))
        const = ctx.enter_context(tc.tile_pool(name="const", bufs=1))
        big = ctx.enter_context(tc.tile_pool(name="big", bufs=1))
        spool = ctx.enter_context(tc.tile_pool(name="spool", bufs=8))
        ebpool = ctx.enter_context(tc.tile_pool(name="ebpool", bufs=10))
        small = ctx.enter_context(tc.tile_pool(name="small", bufs=4))
        drain = ctx.enter_context(tc.tile_pool(name="drain", bufs=3))
        psA = ctx.enter_context(tc.tile_pool(name="psA", bufs=2, space="PSUM"))
        psB = ctx.enter_context(tc.tile_pool(name="psB", bufs=2, space="PSUM"))
        psC = ctx.enter_context(tc.tile_pool(name="psC", bufs=2, space="PSUM"))
        dram = ctx.enter_context(tc.tile_pool(name="dram", bufs=4, space="DRAM"))

        mult = mybir.AluOpType.mult
        add = mybir.AluOpType.add
        amin = mybir.AluOpType.min
        amax = mybir.AluOpType.max
        Act = mybir.ActivationFunctionType

        wkq_sb = const.tile([128, 2, KVP + KD], BF16, tag="wkq")
        wp_sb = const.tile([DV, 4, 128], BF16, tag="wp")
        gb_sb = const.tile([KVP, 6], F32, tag="gb")
        id_sb = const.tile([KVP, DV], BF16, tag="id")
        eps_t = const.tile([128, 1], F32, tag="eps")
        nc.vector.memset(eps_t, EPS)
        ones1_t = const.tile([1, DV], F32, tag="ones1")
        nc.vector.memset(ones1_t, 1.0)

        # v_aug gets its ones column once; the raw-v transposes fill 0:DV.
        v_aug = big.tile([128, B, KT, DV + 1], BF16, tag="vaug")
        nc.gpsimd.memset(v_aug[:, :, :, DV:DV + 1], 1.0)

        xt_sb = big.tile([128, B, 2, NPAD], BF16, tag="xt")
        xs_sb = big.tile([128, B, 2, NQ], BF16, tag="xs")
        y_kv = big.tile([KVP, B, NPAD], BF16, tag="ykv")
        y_q = big.tile([KD, B, NQ], BF16, tag="yq")
        st_kv = small.tile([KVP, 24, 6], F32, tag="st_kv")
        st_q = small.tile([KD, 6, 6], F32, tag="st_q")

        # ------------- pipelined x DMA + kv projection + stats -------------
        def kv_chunk(ch):
            X0, XW = XOF[ch], XCS[ch]
            for b in range(B):
                for c in range(2):
                    nc.sync.dma_start(
                        out=xt_sb[:, b, c, bass.ds(X0, XW)],
                        in_=xT[b, c, :, bass.ds(X0, XW)])
            if ch == 0:
                # weights land between chunk0 and chunk1 transfers
                for c in range(2):
                    nc.sync.dma_start(out=wkq_sb[:, c, :], in_=wkqT[c])
                nc.sync.dma_start(out=wp_sb, in_=wpT[:, :, :])
                nc.sync.dma_start(out=gb_sb, in_=gbT[:, :])
                nc.sync.dma_start(out=id_sb, in_=idT[:, :])
            for b in range(B):
                for u in range(XW // 448):
                    t = (X0 // 448) + u
                    ps = psB.tile([KVP, 448], F32, tag="ps_av")
                    for c in range(2):
                        nc.tensor.matmul(ps, wkq_sb[:, c, 0:KVP],
                                         xt_sb[:, b, c, bass.ds(t * 448, 448)],
                                         start=(c == 0), stop=(c == 1))
                    nc.scalar.copy(out=y_kv[:, b, bass.ds(t * 448, 448)],
                                   in_=ps)
                    # stats off the drained y_kv so the PSUM pipeline is
                    # drain-rate-limited (pads excluded via window width)
                    w = min(448, N - t * 448)
                    nc.vector.bn_stats(out=st_kv[:, t * B + b, :],
                                       in_=y_kv[:, b, bass.ds(t * 448, w)])

        kv_chunk(0)
        kv_chunk(1)
        # xs lands while kv chunk 2 streams
        for b in range(B):
            for c in range(2):
                nc.sync.dma_start(out=xs_sb[:, b, c, :], in_=xsT[b, c])
        kv_chunk(2)
        # q projection slots in while kv chunks 3/4 stream
        for b in range(B):
            for t in range(3):
                ps = psB.tile([KD, 448], F32, tag="ps_av")
                for c in range(2):
                    nc.tensor.matmul(ps, wkq_sb[:, c, KVP:KVP + KD],
                                     xs_sb[:, b, c, bass.ds(t * 448, 448)],
                                     start=(c == 0), stop=(c == 1))
                nc.scalar.copy(out=y_q[:, b, bass.ds(t * 448, 448)], in_=ps)
            for t in range(3):
                nc.vector.bn_stats(out=st_q[:, b * 3 + t, :],
                                   in_=y_q[:, b, bass.ds(t * 448, 448)])
        kv_chunk(3)
        kv_chunk(4)

        # ------------- batch-norm scale/shift + q~ -------------
        def bn_scale_shift(mv, g_ap, b_ap, P, name):
            s = small.tile([P, 1], F32, tag=f"s_{name}")
            t = small.tile([P, 1], F32, tag=f"t_{name}")
            nc.scalar.activation(out=s, in_=mv[:, 1:2], func=Act.Sqrt,
                                 bias=eps_t[0:P])
            nc.vector.reciprocal(out=s, in_=s)
            nc.vector.tensor_mul(s, s, g_ap)
            nc.vector.tensor_mul(t, mv[:, 0:1], s)
            nc.vector.tensor_scalar(out=t, in0=t, scalar1=-1.0, scalar2=None,
                                    op0=mult)
            nc.vector.tensor_add(t, t, b_ap)
            return s, t

        mv_kv = small.tile([KVP, 2], F32, tag="mv_kv")
        nc.vector.bn_aggr(out=mv_kv, in_=st_kv)
        s_kv, t_kv = bn_scale_shift(mv_kv, gb_sb[:, 0:1], gb_sb[:, 1:2],
                                    KVP, "kv")
        mv_q = small.tile([KD, 2], F32, tag="mv_q")
        nc.vector.bn_aggr(out=mv_q, in_=st_q)
        s_q, t_q = bn_scale_shift(mv_q, gb_sb[0:KD, 2:3], gb_sb[0:KD, 3:4],
                                  KD, "q")

        # q~ = (s_k*s_q) . q_raw + s_k*t_q
        a_q = small.tile([KD, 1], F32, tag="a_q")
        b_q = small.tile([KD, 1], F32, tag="b_q")
        nc.vector.tensor_mul(a_q, s_kv[0:KD], s_q)
        nc.vector.tensor_mul(b_q, s_kv[0:KD], t_q)
        qT = big.tile([KD, B, NQ], BF16, tag="qT")
        for b in range(B):
            nc.vector.tensor_scalar(out=qT[:, b, :], in0=y_q[:, b, :],
                                    scalar1=a_q, scalar2=b_q,
                                    op0=mult, op1=add)
        # v-side scale/shift moved to base partition 0 for the drain ops
        s_v0 = small.tile([DV, 1], F32, tag="s_v0")
        t_v0 = small.tile([DV, 1], F32, tag="t_v0")
        nc.gpsimd.dma_start(out=s_v0, in_=s_kv[32:KVP])
        nc.gpsimd.dma_start(out=t_v0, in_=t_kv[32:KVP])
        c3_v = small.tile([DV, 1], F32, tag="c3v")
        nc.vector.tensor_scalar(out=c3_v, in0=t_v0, scalar1=3.0,
                                scalar2=None, op0=add)
        # raw v -> token-major via PE transposes (the DMA transpose engine
        # serializes against later DMAs in the tile scheduler, so avoid it)
        for b in range(B):
            for t0 in range(0, KT, 32):
                nt = min(32, KT - t0)
                pc_t = psC.tile([128, 1024], BF16, tag="pc")
                for i in range(nt):
                    nc.tensor.transpose(
                        pc_t[:, bass.ts(i, DV)],
                        y_kv[32:KVP, b, bass.ds((t0 + i) * 128, 128)],
                        id_sb[32:KVP, :])
                nc.vector.tensor_copy(v_aug[:, b, t0:t0 + nt, 0:DV],
                                      pc_t[:, 0:nt * DV])

        # ------------- attention + partial projection -------------
        hsT = big.tile([DV, B, NQ], BF16, tag="hsT")
        y_fin = big.tile([OC, B, NQ], BF16, tag="y_fin")
        y_out = big.tile([OC, B, NQ], F32, tag="y_out")
        y_bncs = []
        y_sls = []
        for i in range(NQC):
            ybnc_t = dram.tile([4, 128, B * QCS[i]], BF16, tag=f"bnc{i}")
            y_bncs.append(ybnc_t)
            ysl_t = dram.tile([OC, B * QCS[i]], BF16, tag=f"ysl{i}")
            y_sls.append(ysl_t)

        ypars = []
        for i in range(NQC):
            ypar_t = big.tile([128, 4, B, QCS[i]], BF16, tag=f"ypar{i}")
            ypars.append(ypar_t)

        def emit_proj_piece(qc, o, tail=False):
            # one <=512-wide partial-projection matmul per (b, out-block)
            W = QCS[qc]
            q0 = QOF[qc]
            for b in range(B):
                pc = psC.tile([128, 512], F32, tag="pc")
                nc.tensor.matmul(pc[:, 0:W], wp_sb[:, o, :],
                                 hsT[:, b, bass.ds(q0, W)],
                                 start=True, stop=True)
                dst = ypars[qc][:, o, b, 0:W]
                if tail and b == 0:
                    nc.scalar.copy(out=dst, in_=pc[:, 0:W])
                else:
                    nc.vector.tensor_copy(dst, pc[:, 0:W])

        def emit_rs(qc, tail=False):
            dma = nc.scalar.dma_start if tail else nc.gpsimd.dma_start
            dma(out=y_bncs[qc].rearrange("o p (b q) -> p o b q", b=B),
                in_=ypars[qc])
            nc.gpsimd.collective_compute(
                "ReduceScatter", add,
                replica_groups=[list(range(NCORES))],
                ins=[y_bncs[qc].opt()],
                outs=[y_sls[qc].opt()])

        def attn_group(g, b, av_t, eb, W, q0):
            qk = psA.tile([128, GRP, 512], F32, tag="qk")
            for i in range(GRP):
                j = g * GRP + i
                nc.tensor.matmul(qk[:, i, 0:W],
                                 y_kv[0:KD, b, bass.ts(j, 128)],
                                 qT[:, b, bass.ds(q0, W)],
                                 start=True, stop=True)
            sp = spool.tile([128, GRP, 512], BF16, tag="sp")
            nc.scalar.activation(out=sp[:, :, 0:W], in_=qk[:, :, 0:W],
                                 func=Act.Exp, scale=SCALE)
            nc.vector.tensor_mul(sp[:, :, 0:W], sp[:, :, 0:W], eb[:, :, 0:W])
            for i in range(GRP):
                j = g * GRP + i
                nc.tensor.matmul(av_t[:, 0:W], v_aug[:, b, j, :],
                                 sp[:, i, 0:W],
                                 start=(j == 0), stop=(j == KT - 1),
                                 skip_group_check=True)

        def drain_chain(b, av_t, W, q0, alt):
            # out = (av*s_v)/denom + t_v, then hardswish.  alt=True parks the
            # accumulator via ACT (tail: both batches drain in parallel).
            av = drain.tile([DV + 1, 512], F32, tag="av_sb")
            if alt:
                nc.scalar.copy(out=av[:, 0:W], in_=av_t[:, 0:W])
            else:
                nc.vector.tensor_copy(av[:, 0:W], av_t[:, 0:W])
            rec = drain.tile([1, 512], F32, tag="rec")
            nc.vector.reciprocal(out=rec[:, 0:W], in_=av[DV:DV + 1, 0:W])
            pc_r = psC.tile([128, 512], F32, tag="pc")
            recb = pc_r[0:DV, :]
            nc.tensor.matmul(recb[:, 0:W], ones1_t, rec[:, 0:W],
                             start=True, stop=True)
            xo = drain.tile([DV, 512], F32, tag="xo")
            nc.vector.scalar_tensor_tensor(
                out=xo[:, 0:W], in0=av[0:DV, 0:W], scalar=s_v0,
                in1=recb[:, 0:W], op0=mult, op1=mult)
            r3 = drain.tile([DV, 512], F32, tag="r3")
            nc.vector.tensor_scalar(out=r3[:, 0:W], in0=xo[:, 0:W],
                                    scalar1=c3_v, scalar2=0.0,
                                    op0=add, op1=amax)
            nc.vector.tensor_scalar(out=r3[:, 0:W], in0=r3[:, 0:W],
                                    scalar1=6.0, scalar2=1.0 / 6.0,
                                    op0=amin, op1=mult)
            nc.vector.scalar_tensor_tensor(
                out=hsT[:, b, bass.ds(q0, W)], in0=xo[:, 0:W],
                scalar=t_v0, in1=r3[:, 0:W], op0=add, op1=mult)

        for qc in range(NQC - 1):
            W = QCS[qc]
            q0 = QOF[qc]
            avs = []
            for _b in range(B):
                av_t = psB.tile([DV + 1, 512], F32, tag="ps_av")
                avs.append(av_t)
            for g in range(NGRP):
                eb = ebpool.tile([128, GRP, 512], BF16, tag="eb")
                nc.sync.dma_start(out=eb[:, :, 0:W],
                                  in_=ebT[g, :, :, bass.ds(q0, W)])
                if qc > 0 and 5 <= g < 13 and g % 2 == 1:
                    emit_proj_piece(qc - 1, (g - 5) // 2)
                if qc > 0 and g == 13:
                    emit_rs(qc - 1)
                for b in range(B):
                    attn_group(g, b, avs[b], eb, W, q0)
            for b in range(B):
                drain_chain(b, avs[b], W, q0, False)
        # ---- tail chunk, b-outer: b0's drain/proj/bounce hide under b1 ----
        qc = NQC - 1
        W = QCS[qc]
        q0 = QOF[qc]
        for b in range(B):
            av_t = psB.tile([DV + 1, 512], F32, tag="ps_av")
            for g in range(NGRP):
                eb = ebpool.tile([128, GRP, 512], BF16, tag="eb")
                nc.sync.dma_start(out=eb[:, :, 0:W],
                                  in_=ebT[g, :, :, bass.ds(q0, W)])
                if b == 0 and 5 <= g < 13 and g % 2 == 1:
                    emit_proj_piece(qc - 1, (g - 5) // 2)
                if b == 0 and g == 13:
                    emit_rs(qc - 1)
                attn_group(g, b, av_t, eb, W, q0)
            drain_chain(b, av_t, W, q0, alt=(b == 1))
            for o in range(4):
                pc = psC.tile([128, 512], F32, tag="pc")
                nc.tensor.matmul(pc[:, 0:W], wp_sb[:, o, :],
                                 hsT[:, b, bass.ds(q0, W)],
                                 start=True, stop=True)
                dst = ypars[qc][:, o, b, 0:W]
                if b == 0:
                    nc.vector.tensor_copy(dst, pc[:, 0:W])
                else:
                    nc.scalar.copy(out=dst, in_=pc[:, 0:W])
            half_dma = nc.gpsimd.dma_start
            half_dma(
                out=y_bncs[qc][:, :, bass.ds(b * W, W)].rearrange(
                    "o p q -> p o q"),
                in_=ypars[qc][:, :, b, :])
        nc.gpsimd.collective_compute(
            "ReduceScatter", add,
            replica_groups=[list(range(NCORES))],
            ins=[y_bncs[qc].opt()],
            outs=[y_sls[qc].opt()])
        # preload the Sqrt activation table while the collective runs
        sq_warm = small.tile([1, 1], F32, tag="sq_warm")
        nc.scalar.activation(out=sq_warm, in_=eps_t[0:1, :], func=Act.Sqrt,
                             bias=eps_t[0:1])

        if debug:
            nc.sync.dma_start(out=dbg["dbg_ykv"][:, :, :], in_=y_kv)
            nc.sync.dma_start(out=dbg["dbg_qT"][:, :, :], in_=qT)
            nc.sync.dma_start(out=dbg["dbg_vaug"][:, :, :, :], in_=v_aug)
            nc.sync.dma_start(out=dbg["dbg_hsT"][:, :, :], in_=hsT)
            nc.sync.dma_start(out=dbg["dbg_mvkv"][:, :], in_=mv_kv)

        # ------------- gather slices + final BN (kept off queue heads) -----
        ctx.enter_context(tc.tile_wait_until(0.25))
        st_p = small.tile([OC, B * NQC, 6], F32, tag="st_p")
        for qc in range(NQC):
            W = QCS[qc]
            for b in range(B):
                nc.sync.dma_start(
                    out=y_fin[:, b, bass.ds(QOF[qc], W)],
                    in_=y_sls[qc][:, bass.ds(b * W, W)])
                nc.vector.bn_stats(out=st_p[:, qc * B + b, :],
                                   in_=y_fin[:, b, bass.ds(QOF[qc], W)])
        mv_p = small.tile([OC, 2], F32, tag="mv_p")
        nc.vector.bn_aggr(out=mv_p, in_=st_p)
        s_p, t_p = bn_scale_shift(mv_p, gb_sb[:, 4:5], gb_sb[:, 5:6],
                                  OC, "p")
        nc.vector.tensor_scalar(out=y_out[:, 0, :], in0=y_fin[:, 0, :],
                                scalar1=s_p, scalar2=t_p, op0=mult, op1=add)
        nc.vector.tensor_scalar(out=y_out[:, 1, :], in0=y_fin[:, 1, :],
                                scalar1=s_p, scalar2=t_p, op0=mult, op1=add)
        if debug:
            nc.sync.dma_start(out=dbg["dbg_yfin"][:, :, :], in_=y_fin)
        nc.sync.dma_start(out=yT[:, 0:NQ], in_=y_out[:, 0, :])
        nc.sync.dma_start(out=yT[:, NQ:2 * NQ], in_=y_out[:, 1, :])

    nc.finalize()
    return nc


def _prep_inputs(x, kv_w, kv_g, kv_b, q_w, q_g, q_b, proj_w, proj_g, proj_b,
                 bias_table, bias_idxs):
    """Host-side sharding/layout prep. Returns list of 8 per-core input maps."""
    x = np.asarray(x, np.float32)
    xt = np.zeros((B, 2, 128, NPAD), np.float32)
    xTt = x.transpose(0, 2, 1)  # (B, 256, N)
    xt[:, :, :, :N] = xTt.reshape(B, 2, 128, N)
    xt = xt.astype(bf16)

    # exp(bias) tables per head, padded-k zeroed, laid out (NGRP, 128, GRP, NQ)
    rank2 = np.asarray(bias_idxs)[0].reshape(ROW, COL)  # (dr, dc) -> id
    table2 = np.asarray(bias_table, np.float32)[:, rank2]  # (H, 63, 84)
    eb2 = np.exp(table2)
    kk = np.arange(N)
    qq = np.arange(NQ)
    DRm = np.abs(kk[:, None] // COL - 2 * (qq[None, :] // COL_))
    DCm = np.abs(kk[:, None] % COL - 2 * (qq[None, :] % COL_))

    xs = x.reshape(B, ROW, COL, CIN)[:, ::2, ::2].reshape(B, NQ, CIN)
    xst = xs.transpose(0, 2, 1).reshape(B, 2, 128, NQ).astype(bf16)
    in_maps = []
    for h in range(H):
        ebf = np.zeros((NPAD, NQ), np.float32)
        ebf[:N] = eb2[h][DRm, DCm]
        ebl = np.ascontiguousarray(
            ebf.reshape(NGRP, GRP, 128, NQ).transpose(0, 2, 1, 3)
        ).astype(bf16)
        sl = slice(h * HKV, (h + 1) * HKV)
        slq = slice(h * KD, (h + 1) * KD)
        slo = slice(h * OC, (h + 1) * OC)
        wkv_pad = np.zeros((KVP, CIN), np.float32)
        wkv_pad[0:KD] = np.asarray(kv_w, np.float32)[sl][0:KD]
        wkv_pad[32:KVP] = np.asarray(kv_w, np.float32)[sl][KD:HKV]
        # combined kv+q weights: (2, 128, KVP+KD)
        wkq = np.zeros((CIN, KVP + KD), np.float32)
        wkq[:, 0:KVP] = wkv_pad.T
        wkq[:, KVP:] = np.asarray(q_w, np.float32)[slq].T
        # packed gains/biases: kv at [:,0:2], q at rows 0:16 [:,2:4],
        # proj at [:,4:6]
        gb = np.zeros((KVP, 6), np.float32)
        gb[:, 0] = 1.0
        gb[0:KD, 0] = np.asarray(kv_g, np.float32)[sl][0:KD]
        gb[0:KD, 1] = np.asarray(kv_b, np.float32)[sl][0:KD]
        gb[32:KVP, 0] = np.asarray(kv_g, np.float32)[sl][KD:HKV]
        gb[32:KVP, 1] = np.asarray(kv_b, np.float32)[sl][KD:HKV]
        gb[0:KD, 2] = np.asarray(q_g, np.float32)[slq]
        gb[0:KD, 3] = np.asarray(q_b, np.float32)[slq]
        gb[:, 4] = np.asarray(proj_g, np.float32)[slo]
        gb[:, 5] = np.asarray(proj_b, np.float32)[slo]
        # W_p columns for this head's 32 v-channels, as 4 stationary tiles
        wp_h = np.asarray(proj_w, np.float32)[:, h * DV:(h + 1) * DV]
        wp_t = np.ascontiguousarray(wp_h.T.reshape(DV, 4, 128)).astype(bf16)
        in_maps.append({
            "xT": xt,
            "xsT": xst,
            "idT": np.concatenate([np.zeros((DV, DV), np.float32), np.eye(DV, dtype=np.float32)]).astype(bf16),
            "wkqT": np.ascontiguousarray(
                wkq.reshape(2, 128, KVP + KD)).astype(bf16),
            "wpT": wp_t,
            "gbT": np.ascontiguousarray(gb),
            "ebT": ebl,
        })
    return in_maps


def kernel(x, kv_w, kv_g, kv_b, q_w, q_g, q_b, proj_w, proj_g, proj_b,
           bias_table, bias_idxs, _trace=False):
    global LAST_EXEC_NS
    if "nc" not in _prog_cache:
        _prog_cache["nc"] = _build_program()
    nc = _prog_cache["nc"]
    in_maps = _prep_inputs(x, kv_w, kv_g, kv_b, q_w, q_g, q_b,
                           proj_w, proj_g, proj_b, bias_table, bias_idxs)
    res = run_bass_kernel_spmd(nc, in_maps, core_ids=list(range(NCORES)),
                               trace=_trace)
    LAST_EXEC_NS = res.exec_time_ns
    yts = [np.asarray(r["yT"]) for r in res.results]  # each (OC, B*NQ)
    y = np.concatenate(yts, axis=0)                   # (512, B*NQ)
    return np.ascontiguousarray(
        y.T.reshape(B, NQ, H * OC).astype(np.float32))


# revision 58
# speedup vs baseline: 1.0091x; 1.0010x over previous
"""AttentionSubsample kernel for 8 trn2 NeuronCores.

Sharding: head-parallel (8 heads -> 8 cores). Each core runs its head through
kv/q projection, attention and hardswish, then computes a PARTIAL output
projection (512 out channels x its 32 v-channels) which is summed across
cores with a per-q-chunk ReduceScatter; core i keeps output rows 64i:64i+64,
so the final BatchNorm is purely local.

Schedule notes:
- x is DMA'd in 4 token chunks; kv projection, PSUM-side bn_stats and drains
  trail each chunk so the kv-stat barrier lands right after the last DMA.
- BN folding: the k-side BatchNorm shift cancels inside the softmax (it is
  constant per query), so QK runs on RAW k with q~ = (s_k*s_q) q_raw +
  s_k*t_q.  The v-side BatchNorm is folded into the post-attention drain:
  out = (av * s_v) * (1/denom) + t_v via scalar_tensor_tensor.
- Attention phase keeps the Activation engine exp-only; all exp(bias)
  multiplies run on DVE (gpsimd multiply efficiency is only 0.42).
- Softmax denominator from a ones-column in the AV stationary (33rd col).
- q chunks are [512, 512, 320]: the final (tail) ReduceScatter is small and
  starts as early as possible.
- The post-collective ops are issued under tile_wait_until so the scheduler
  cannot hoist them into the attention phase (head-of-line queue blocking).
"""

import numpy as np
import ml_dtypes

import concourse.bass as bass
import concourse.mybir as mybir
import concourse.tile as tile
from concourse import bacc
from contextlib import ExitStack
from concourse.bass_utils import run_bass_kernel_spmd

BF16 = mybir.dt.bfloat16
F32 = mybir.dt.float32
bf16 = ml_dtypes.bfloat16
F8 = mybir.dt.float8e4
f8 = ml_dtypes.float8_e4m3

B = 2
ROW, COL = 63, 84
ROW_, COL_ = 32, 42
N = ROW * COL            # 5292 kv tokens
NQ = ROW_ * COL_         # 1344 q tokens
NPAD = 5376              # 42*128 padded kv tokens
KT = NPAD // 128         # 42 k-tiles
QCS = [512, 512, 320]    # q chunks (tail chunk small -> short last collective)
QOF = [0, 512, 1024]
NQC = len(QCS)
CIN = 256
H = 8
KD = 16
DV = 32
HKV = KD + DV            # 48 per-head kv channels
KVP = 64                 # padded kv rows: k at 0:16, v at 32:64
OC = 64                  # per-core slice of the 512 output channels
GRP = 2                  # k-tiles per exp group
NGRP = KT // GRP         # 21
EPS = 1e-5
SCALE = KD ** -0.5
NCORES = 8
XCS = [1344, 1344, 1344, 1344]       # x DMA chunks
XOF = [0, 1344, 2688, 4032]

LAST_EXEC_NS = None
_prog_cache = {}


def _build_program(debug=False):
    nc = bacc.Bacc(num_devices=NCORES)

    xT = nc.dram_tensor("xT", [B, 2, 128, NPAD], BF16, kind="ExternalInput")
    xsT = nc.dram_tensor("xsT", [B, 2, 128, NQ], BF16, kind="ExternalInput")
    wkqT = nc.dram_tensor("wkqT", [2, 128, KVP + KD], BF16,
                          kind="ExternalInput")
    wpT = nc.dram_tensor("wpT", [DV, 4, 128], BF16, kind="ExternalInput")
    gbT = nc.dram_tensor("gbT", [KVP, 6], F32, kind="ExternalInput")
    idT = nc.dram_tensor("idT", [KVP, DV], BF16, kind="ExternalInput")
    ebT = nc.dram_tensor("ebT", [NGRP, 128, GRP, NQ], BF16,
                         kind="ExternalInput")
    yT = nc.dram_tensor("yT", [OC, B * NQ], F32, kind="ExternalOutput")
    if debug:
        dbg = {
            "dbg_ykv": nc.dram_tensor("dbg_ykv", [KVP, B, NPAD], BF16,
                                      kind="ExternalOutput"),
            "dbg_qT": nc.dram_tensor("dbg_qT", [KD, B, NQ], BF16,
                                     kind="ExternalOutput"),
            "dbg_vaug": nc.dram_tensor("dbg_vaug", [128, B, KT, DV + 1], BF16,
                                       kind="ExternalOutput"),
            "dbg_hsT": nc.dram_tensor("dbg_hsT", [DV, B, NQ], BF16,
                                      kind="ExternalOutput"),
            "dbg_mvkv": nc.dram_tensor("dbg_mvkv", [KVP, 2], F32,
                                       kind="ExternalOutput"),
            "dbg_yfin": nc.dram_tensor("dbg_yfin", [OC, B, NQ], BF16,
                                       kind="ExternalOutput"),
        }

    with ExitStack() as ctx:
        tc = ctx.enter_context(tile.TileContext(nc))
        const = ctx.enter_context(tc.tile_pool(name="const", bufs=1))
        big = ctx.enter_context(tc.tile_pool(name="big", bufs=1))
        spool = ctx.enter_context(tc.tile_pool(name="spool", bufs=8))
        ebpool = ctx.enter_context(tc.tile_pool(name="ebpool", bufs=10))
        small = ctx.enter_context(tc.tile_pool(name="small", bufs=4))
        drain = ctx.enter_context(tc.tile_pool(name="drain", bufs=3))
        psA = ctx.enter_context(tc.tile_pool(name="psA", bufs=2, space="PSUM"))
        psB = ctx.enter_context(tc.tile_pool(name="psB", bufs=2, space="PSUM"))
        psC = ctx.enter_context(tc.tile_pool(name="psC", bufs=2, space="PSUM"))
        dram = ctx.enter_context(tc.tile_pool(name="dram", bufs=4, space="DRAM"))

        mult = mybir.AluOpType.mult
        add = mybir.AluOpType.add
        amin = mybir.AluOpType.min
        amax = mybir.AluOpType.max
        Act = mybir.ActivationFunctionType

        wkq_sb = const.tile([128, 2, KVP + KD], BF16, tag="wkq")
        wp_sb = const.tile([DV, 4, 128], BF16, tag="wp")
        gb_sb = const.tile([KVP, 6], F32, tag="gb")
        id_sb = const.tile([KVP, DV], BF16, tag="id")
        eps_t = const.tile([128, 1], F32, tag="eps")
        nc.vector.memset(eps_t, EPS)
        ones1_t = const.tile([1, DV], F32, tag="ones1")
        nc.vector.memset(ones1_t, 1.0)

        # v_aug gets its ones column once; the raw-v transposes fill 0:DV.
        v_aug = big.tile([128, B, KT, DV + 1], BF16, tag="vaug")
        nc.gpsimd.memset(v_aug[:, :, :, DV:DV + 1], 1.0)

        xt_sb = big.tile([128, B, 2, NPAD], BF16, tag="xt")
        xs_sb = big.tile([128, B, 2, NQ], BF16, tag="xs")
        y_kv = big.tile([KVP, B, NPAD], BF16, tag="ykv")
        y_q = big.tile([KD, B, NQ], BF16, tag="yq")
        st_kv = small.tile([KVP, 24, 6], F32, tag="st_kv")
        st_q = small.tile([KD, 6, 6], F32, tag="st_q")

        # ------------- pipelined x DMA + kv projection + stats -------------
        def kv_chunk(ch):
            X0, XW = XOF[ch], XCS[ch]
            for b in range(B):
                for c in range(2):
                    nc.sync.dma_start(
                        out=xt_sb[:, b, c, bass.ds(X0, XW)],
                        in_=xT[b, c, :, bass.ds(X0, XW)])
            if ch == 0:
                # weights land between chunk0 and chunk1 transfers
                for c in range(2):
                    nc.sync.dma_start(out=wkq_sb[:, c, :], in_=wkqT[c])
                nc.sync.dma_start(out=wp_sb, in_=wpT[:, :, :])
                nc.sync.dma_start(out=gb_sb, in_=gbT[:, :])
                nc.sync.dma_start(out=id_sb, in_=idT[:, :])
            for b in range(B):
                for u in range(XW // 448):
                    t = (X0 // 448) + u
                    ps = psB.tile([KVP, 448], F32, tag="ps_av")
                    for c in range(2):
                        nc.tensor.matmul(ps, wkq_sb[:, c, 0:KVP],
                                         xt_sb[:, b, c, bass.ds(t * 448, 448)],
                                         start=(c == 0), stop=(c == 1))
                    nc.scalar.copy(out=y_kv[:, b, bass.ds(t * 448, 448)],
                                   in_=ps)
                    # stats off the drained y_kv so the PSUM pipeline is
                    # drain-rate-limited (pads excluded via window width)
                    w = min(448, N - t * 448)
                    nc.vector.bn_stats(out=st_kv[:, t * B + b, :],
                                       in_=y_kv[:, b, bass.ds(t * 448, w)])

        kv_chunk(0)
        kv_chunk(1)
        # xs lands while kv chunk 2 streams
        for b in range(B):
            for c in range(2):
                nc.sync.dma_start(out=xs_sb[:, b, c, :], in_=xsT[b, c])
        kv_chunk(2)
        # q projection slots in while kv chunks 3/4 stream
        for b in range(B):
            for t in range(3):
                ps = psB.tile([KD, 448], F32, tag="ps_av")
                for c in range(2):
                    nc.tensor.matmul(ps, wkq_sb[:, c, KVP:KVP + KD],
                                     xs_sb[:, b, c, bass.ds(t * 448, 448)],
                                     start=(c == 0), stop=(c == 1))
                nc.scalar.copy(out=y_q[:, b, bass.ds(t * 448, 448)], in_=ps)
            for t in range(3):
                nc.vector.bn_stats(out=st_q[:, b * 3 + t, :],
                                   in_=y_q[:, b, bass.ds(t * 448, 448)])
        kv_chunk(3)

        # ------------- batch-norm scale/shift + q~ -------------
        def bn_scale_shift(mv, g_ap, b_ap, P, name):
            s = small.tile([P, 1], F32, tag=f"s_{name}")
            t = small.tile([P, 1], F32, tag=f"t_{name}")
            nc.scalar.activation(out=s, in_=mv[:, 1:2], func=Act.Sqrt,
                                 bias=eps_t[0:P])
            nc.vector.reciprocal(out=s, in_=s)
            nc.vector.tensor_mul(s, s, g_ap)
            nc.vector.tensor_mul(t, mv[:, 0:1], s)
            nc.vector.tensor_scalar(out=t, in0=t, scalar1=-1.0, scalar2=None,
                                    op0=mult)
            nc.vector.tensor_add(t, t, b_ap)
            return s, t

        mv_kv = small.tile([KVP, 2], F32, tag="mv_kv")
        nc.vector.bn_aggr(out=mv_kv, in_=st_kv)
        s_kv, t_kv = bn_scale_shift(mv_kv, gb_sb[:, 0:1], gb_sb[:, 1:2],
                                    KVP, "kv")
        mv_q = small.tile([KD, 2], F32, tag="mv_q")
        nc.vector.bn_aggr(out=mv_q, in_=st_q)
        s_q, t_q = bn_scale_shift(mv_q, gb_sb[0:KD, 2:3], gb_sb[0:KD, 3:4],
                                  KD, "q")

        # q~ = (s_k*s_q) . q_raw + s_k*t_q
        a_q = small.tile([KD, 1], F32, tag="a_q")
        b_q = small.tile([KD, 1], F32, tag="b_q")
        nc.vector.tensor_mul(a_q, s_kv[0:KD], s_q)
        nc.vector.tensor_mul(b_q, s_kv[0:KD], t_q)
        qT = big.tile([KD, B, NQ], BF16, tag="qT")
        for b in range(B):
            nc.vector.tensor_scalar(out=qT[:, b, :], in0=y_q[:, b, :],
                                    scalar1=a_q, scalar2=b_q,
                                    op0=mult, op1=add)
        # v-side scale/shift moved to base partition 0 for the drain ops
        s_v0 = small.tile([DV, 1], F32, tag="s_v0")
        t_v0 = small.tile([DV, 1], F32, tag="t_v0")
        nc.gpsimd.dma_start(out=s_v0, in_=s_kv[32:KVP])
        nc.gpsimd.dma_start(out=t_v0, in_=t_kv[32:KVP])
        c3_v = small.tile([DV, 1], F32, tag="c3v")
        nc.vector.tensor_scalar(out=c3_v, in0=t_v0, scalar1=3.0,
                                scalar2=None, op0=add)
        # raw v -> token-major via PE transposes (the DMA transpose engine
        # serializes against later DMAs in the tile scheduler, so avoid it)
        for b in range(B):
            for t0 in range(0, KT, 32):
                nt = min(32, KT - t0)
                pc_t = psC.tile([128, 1024], BF16, tag="pc")
                for i in range(nt):
                    nc.tensor.transpose(
                        pc_t[:, bass.ts(i, DV)],
                        y_kv[32:KVP, b, bass.ds((t0 + i) * 128, 128)],
                        id_sb[32:KVP, :])
                nc.vector.tensor_copy(v_aug[:, b, t0:t0 + nt, 0:DV],
                                      pc_t[:, 0:nt * DV])

        # ------------- attention + partial projection -------------
        hsT = big.tile([DV, B, NQ], BF16, tag="hsT")
        y_fin = big.tile([OC, B, NQ], BF16, tag="y_fin")
        y_out = big.tile([OC, B, NQ], F32, tag="y_out")
        y_bncs = []
        y_sls = []
        for i in range(NQC):
            ybnc_t = dram.tile([4, 128, B * QCS[i]], BF16, tag=f"bnc{i}")
            y_bncs.append(ybnc_t)
            ysl_t = dram.tile([OC, B * QCS[i]], BF16, tag=f"ysl{i}")
            y_sls.append(ysl_t)

        ypars = []
        for i in range(NQC):
            ypar_t = big.tile([128, 4, B, QCS[i]], BF16, tag=f"ypar{i}")
            ypars.append(ypar_t)

        def emit_proj_piece(qc, o, tail=False):
            # one <=512-wide partial-projection matmul per (b, out-block)
            W = QCS[qc]
            q0 = QOF[qc]
            for b in range(B):
                pc = psC.tile([128, 512], F32, tag="pc")
                nc.tensor.matmul(pc[:, 0:W], wp_sb[:, o, :],
                                 hsT[:, b, bass.ds(q0, W)],
                                 start=True, stop=True)
                dst = ypars[qc][:, o, b, 0:W]
                if tail and b == 0:
                    nc.scalar.copy(out=dst, in_=pc[:, 0:W])
                else:
                    nc.vector.tensor_copy(dst, pc[:, 0:W])

        def emit_rs(qc, tail=False):
            dma = nc.scalar.dma_start if tail else nc.gpsimd.dma_start
            dma(out=y_bncs[qc].rearrange("o p (b q) -> p o b q", b=B),
                in_=ypars[qc])
            nc.gpsimd.collective_compute(
                "ReduceScatter", add,
                replica_groups=[list(range(NCORES))],
                ins=[y_bncs[qc].opt()],
                outs=[y_sls[qc].opt()])

        def attn_group(g, b, av_t, eb, W, q0):
            qk = psA.tile([128, GRP, 512], F32, tag="qk")
            for i in range(GRP):
                j = g * GRP + i
                nc.tensor.matmul(qk[:, i, 0:W],
                                 y_kv[0:KD, b, bass.ts(j, 128)],
                                 qT[:, b, bass.ds(q0, W)],
                                 start=True, stop=True)
            sp = spool.tile([128, GRP, 512], BF16, tag="sp")
            nc.scalar.activation(out=sp[:, :, 0:W], in_=qk[:, :, 0:W],
                                 func=Act.Exp, scale=SCALE)
            nc.vector.tensor_mul(sp[:, :, 0:W], sp[:, :, 0:W], eb[:, :, 0:W])
            for i in range(GRP):
                j = g * GRP + i
                nc.tensor.matmul(av_t[:, 0:W], v_aug[:, b, j, :],
                                 sp[:, i, 0:W],
                                 start=(j == 0), stop=(j == KT - 1),
                                 skip_group_check=True)

        def drain_chain(b, av_t, W, q0, alt):
            # out = (av*s_v)/denom + t_v, then hardswish.  alt=True parks the
            # accumulator via ACT (tail: both batches drain in parallel).
            av = drain.tile([DV + 1, 512], F32, tag="av_sb")
            if alt:
                nc.scalar.copy(out=av[:, 0:W], in_=av_t[:, 0:W])
            else:
                nc.vector.tensor_copy(av[:, 0:W], av_t[:, 0:W])
            rec = drain.tile([1, 512], F32, tag="rec")
            nc.vector.reciprocal(out=rec[:, 0:W], in_=av[DV:DV + 1, 0:W])
            pc_r = psC.tile([128, 512], F32, tag="pc")
            recb = pc_r[0:DV, :]
            nc.tensor.matmul(recb[:, 0:W], ones1_t, rec[:, 0:W],
                             start=True, stop=True)
            xo = drain.tile([DV, 512], F32, tag="xo")
            nc.vector.scalar_tensor_tensor(
                out=xo[:, 0:W], in0=av[0:DV, 0:W], scalar=s_v0,
                in1=recb[:, 0:W], op0=mult, op1=mult)
            r3 = drain.tile([DV, 512], F32, tag="r3")
            nc.vector.tensor_scalar(out=r3[:, 0:W], in0=xo[:, 0:W],
                                    scalar1=c3_v, scalar2=0.0,
                                    op0=add, op1=amax)
            nc.vector.tensor_scalar(out=r3[:, 0:W], in0=r3[:, 0:W],
                                    scalar1=6.0, scalar2=1.0 / 6.0,
                                    op0=amin, op1=mult)
            nc.vector.scalar_tensor_tensor(
                out=hsT[:, b, bass.ds(q0, W)], in0=xo[:, 0:W],
                scalar=t_v0, in1=r3[:, 0:W], op0=add, op1=mult)

        for qc in range(NQC - 1):
            W = QCS[qc]
            q0 = QOF[qc]
            avs = []
            for _b in range(B):
                av_t = psB.tile([DV + 1, 512], F32, tag="ps_av")
                avs.append(av_t)
            for g in range(NGRP):
                eb = ebpool.tile([128, GRP, 512], BF16, tag="eb")
                nc.sync.dma_start(out=eb[:, :, 0:W],
                                  in_=ebT[g, :, :, bass.ds(q0, W)])
                if qc > 0 and 5 <= g < 13 and g % 2 == 1:
                    emit_proj_piece(qc - 1, (g - 5) // 2)
                if qc > 0 and g == 13:
                    emit_rs(qc - 1)
                for b in range(B):
                    attn_group(g, b, avs[b], eb, W, q0)
            for b in range(B):
                drain_chain(b, avs[b], W, q0, False)
        # ---- tail chunk, b-outer: b0's drain/proj/bounce hide under b1 ----
        qc = NQC - 1
        W = QCS[qc]
        q0 = QOF[qc]
        for b in range(B):
            av_t = psB.tile([DV + 1, 512], F32, tag="ps_av")
            for g in range(NGRP):
                eb = ebpool.tile([128, GRP, 512], BF16, tag="eb")
                nc.sync.dma_start(out=eb[:, :, 0:W],
                                  in_=ebT[g, :, :, bass.ds(q0, W)])
                if b == 0 and 5 <= g < 13 and g % 2 == 1:
                    emit_proj_piece(qc - 1, (g - 5) // 2)
                if b == 0 and g == 13:
                    emit_rs(qc - 1)
                attn_group(g, b, av_t, eb, W, q0)
            drain_chain(b, av_t, W, q0, alt=(b == 1))
            for o in range(4):
                pc = psC.tile([128, 512], F32, tag="pc")
                nc.tensor.matmul(pc[:, 0:W], wp_sb[:, o, :],
                                 hsT[:, b, bass.ds(q0, W)],
                                 start=True, stop=True)
                dst = ypars[qc][:, o, b, 0:W]
                if b == 0:
                    nc.vector.tensor_copy(dst, pc[:, 0:W])
                else:
                    nc.scalar.copy(out=dst, in_=pc[:, 0:W])
            half_dma = nc.gpsimd.dma_start
            half_dma(
                out=y_bncs[qc][:, :, bass.ds(b * W, W)].rearrange(
                    "o p q -> p o q"),
                in_=ypars[qc][:, :, b, :])
        nc.gpsimd.collective_compute(
            "ReduceScatter", add,
            replica_groups=[list(range(NCORES))],
            ins=[y_bncs[qc].opt()],
            outs=[y_sls[qc].opt()])
        # preload the Sqrt activation table while the collective runs
        sq_warm = small.tile([1, 1], F32, tag="sq_warm")
        nc.scalar.activation(out=sq_warm, in_=eps_t[0:1, :], func=Act.Sqrt,
                             bias=eps_t[0:1])

        if debug:
            nc.sync.dma_start(out=dbg["dbg_ykv"][:, :, :], in_=y_kv)
            nc.sync.dma_start(out=dbg["dbg_qT"][:, :, :], in_=qT)
            nc.sync.dma_start(out=dbg["dbg_vaug"][:, :, :, :], in_=v_aug)
            nc.sync.dma_start(out=dbg["dbg_hsT"][:, :, :], in_=hsT)
            nc.sync.dma_start(out=dbg["dbg_mvkv"][:, :], in_=mv_kv)

        # ------------- gather slices + final BN (kept off queue heads) -----
        ctx.enter_context(tc.tile_wait_until(0.25))
        st_p = small.tile([OC, B * NQC, 6], F32, tag="st_p")
        for qc in range(NQC):
            W = QCS[qc]
            for b in range(B):
                nc.sync.dma_start(
                    out=y_fin[:, b, bass.ds(QOF[qc], W)],
                    in_=y_sls[qc][:, bass.ds(b * W, W)])
                nc.vector.bn_stats(out=st_p[:, qc * B + b, :],
                                   in_=y_fin[:, b, bass.ds(QOF[qc], W)])
        mv_p = small.tile([OC, 2], F32, tag="mv_p")
        nc.vector.bn_aggr(out=mv_p, in_=st_p)
        s_p, t_p = bn_scale_shift(mv_p, gb_sb[:, 4:5], gb_sb[:, 5:6],
                                  OC, "p")
        nc.vector.tensor_scalar(out=y_out[:, 0, :], in0=y_fin[:, 0, :],
                                scalar1=s_p, scalar2=t_p, op0=mult, op1=add)
        nc.vector.tensor_scalar(out=y_out[:, 1, :], in0=y_fin[:, 1, :],
                                scalar1=s_p, scalar2=t_p, op0=mult, op1=add)
        if debug:
            nc.sync.dma_start(out=dbg["dbg_yfin"][:, :, :], in_=y_fin)
        nc.sync.dma_start(out=yT[:, 0:NQ], in_=y_out[:, 0, :])
        nc.sync.dma_start(out=yT[:, NQ:2 * NQ], in_=y_out[:, 1, :])

    nc.finalize()
    return nc


def _prep_inputs(x, kv_w, kv_g, kv_b, q_w, q_g, q_b, proj_w, proj_g, proj_b,
                 bias_table, bias_idxs):
    """Host-side sharding/layout prep. Returns list of 8 per-core input maps."""
    x = np.asarray(x, np.float32)
    xt = np.zeros((B, 2, 128, NPAD), np.float32)
    xTt = x.transpose(0, 2, 1)  # (B, 256, N)
    xt[:, :, :, :N] = xTt.reshape(B, 2, 128, N)
    xt = xt.astype(bf16)

    # exp(bias) tables per head, padded-k zeroed, laid out (NGRP, 128, GRP, NQ)
    rank2 = np.asarray(bias_idxs)[0].reshape(ROW, COL)  # (dr, dc) -> id
    table2 = np.asarray(bias_table, np.float32)[:, rank2]  # (H, 63, 84)
    eb2 = np.exp(table2)
    kk = np.arange(N)
    qq = np.arange(NQ)
    DRm = np.abs(kk[:, None] // COL - 2 * (qq[None, :] // COL_))
    DCm = np.abs(kk[:, None] % COL - 2 * (qq[None, :] % COL_))

    xs = x.reshape(B, ROW, COL, CIN)[:, ::2, ::2].reshape(B, NQ, CIN)
    xst = xs.transpose(0, 2, 1).reshape(B, 2, 128, NQ).astype(bf16)
    in_maps = []
    for h in range(H):
        ebf = np.zeros((NPAD, NQ), np.float32)
        ebf[:N] = eb2[h][DRm, DCm]
        ebl = np.ascontiguousarray(
            ebf.reshape(NGRP, GRP, 128, NQ).transpose(0, 2, 1, 3)
        ).astype(bf16)
        sl = slice(h * HKV, (h + 1) * HKV)
        slq = slice(h * KD, (h + 1) * KD)
        slo = slice(h * OC, (h + 1) * OC)
        wkv_pad = np.zeros((KVP, CIN), np.float32)
        wkv_pad[0:KD] = np.asarray(kv_w, np.float32)[sl][0:KD]
        wkv_pad[32:KVP] = np.asarray(kv_w, np.float32)[sl][KD:HKV]
        # combined kv+q weights: (2, 128, KVP+KD)
        wkq = np.zeros((CIN, KVP + KD), np.float32)
        wkq[:, 0:KVP] = wkv_pad.T
        wkq[:, KVP:] = np.asarray(q_w, np.float32)[slq].T
        # packed gains/biases: kv at [:,0:2], q at rows 0:16 [:,2:4],
        # proj at [:,4:6]
        gb = np.zeros((KVP, 6), np.float32)
        gb[:, 0] = 1.0
        gb[0:KD, 0] = np.asarray(kv_g, np.float32)[sl][0:KD]
        gb[0:KD, 1] = np.asarray(kv_b, np.float32)[sl][0:KD]
        gb[32:KVP, 0] = np.asarray(kv_g, np.float32)[sl][KD:HKV]
        gb[32:KVP, 1] = np.asarray(kv_b, np.float32)[sl][KD:HKV]
        gb[0:KD, 2] = np.asarray(q_g, np.float32)[slq]
        gb[0:KD, 3] = np.asarray(q_b, np.float32)[slq]
        gb[:, 4] = np.asarray(proj_g, np.float32)[slo]
        gb[:, 5] = np.asarray(proj_b, np.float32)[slo]
        # W_p columns for this head's 32 v-channels, as 4 stationary tiles
        wp_h = np.asarray(proj_w, np.float32)[:, h * DV:(h + 1) * DV]
        wp_t = np.ascontiguousarray(wp_h.T.reshape(DV, 4, 128)).astype(bf16)
        in_maps.append({
            "xT": xt,
            "xsT": xst,
            "idT": np.concatenate([np.zeros((DV, DV), np.float32), np.eye(DV, dtype=np.float32)]).astype(bf16),
            "wkqT": np.ascontiguousarray(
                wkq.reshape(2, 128, KVP + KD)).astype(bf16),
            "wpT": wp_t,
            "gbT": np.ascontiguousarray(gb),
            "ebT": ebl,
        })
    return in_maps


def kernel(x, kv_w, kv_g, kv_b, q_w, q_g, q_b, proj_w, proj_g, proj_b,
           bias_table, bias_idxs, _trace=False):
    global LAST_EXEC_NS
    if "nc" not in _prog_cache:
        _prog_cache["nc"] = _build_program()
    nc = _prog_cache["nc"]
    in_maps = _prep_inputs(x, kv_w, kv_g, kv_b, q_w, q_g, q_b,
                           proj_w, proj_g, proj_b, bias_table, bias_idxs)
    res = run_bass_kernel_spmd(nc, in_maps, core_ids=list(range(NCORES)),
                               trace=_trace)
    LAST_EXEC_NS = res.exec_time_ns
    yts = [np.asarray(r["yT"]) for r in res.results]  # each (OC, B*NQ)
    y = np.concatenate(yts, axis=0)                   # (512, B*NQ)
    return np.ascontiguousarray(
        y.T.reshape(B, NQ, H * OC).astype(np.float32))


# revision 59
# speedup vs baseline: 1.0152x; 1.0061x over previous
"""AttentionSubsample kernel for 8 trn2 NeuronCores.

Sharding: head-parallel (8 heads -> 8 cores). Each core runs its head through
kv/q projection, attention and hardswish, then computes a PARTIAL output
projection (512 out channels x its 32 v-channels) which is summed across
cores with a per-q-chunk ReduceScatter; core i keeps output rows 64i:64i+64,
so the final BatchNorm is purely local.

Schedule notes:
- x is DMA'd in 4 token chunks; kv projection, PSUM-side bn_stats and drains
  trail each chunk so the kv-stat barrier lands right after the last DMA.
- BN folding: the k-side BatchNorm shift cancels inside the softmax (it is
  constant per query), so QK runs on RAW k with q~ = (s_k*s_q) q_raw +
  s_k*t_q.  The v-side BatchNorm is folded into the post-attention drain:
  out = (av * s_v) * (1/denom) + t_v via scalar_tensor_tensor.
- Attention phase keeps the Activation engine exp-only; all exp(bias)
  multiplies run on DVE (gpsimd multiply efficiency is only 0.42).
- Softmax denominator from a ones-column in the AV stationary (33rd col).
- q chunks are [512, 512, 320]: the final (tail) ReduceScatter is small and
  starts as early as possible.
- The post-collective ops are issued under tile_wait_until so the scheduler
  cannot hoist them into the attention phase (head-of-line queue blocking).
"""

import numpy as np
import ml_dtypes

import concourse.bass as bass
import concourse.mybir as mybir
import concourse.tile as tile
from concourse import bacc
from contextlib import ExitStack
from concourse.bass_utils import run_bass_kernel_spmd

BF16 = mybir.dt.bfloat16
F32 = mybir.dt.float32
bf16 = ml_dtypes.bfloat16
F8 = mybir.dt.float8e4
f8 = ml_dtypes.float8_e4m3

B = 2
ROW, COL = 63, 84
ROW_, COL_ = 32, 42
N = ROW * COL            # 5292 kv tokens
NQ = ROW_ * COL_         # 1344 q tokens
NPAD = 5376              # 42*128 padded kv tokens
KT = NPAD // 128         # 42 k-tiles
QCS = [512, 448, 384]    # q chunks (tail chunk small -> short last collective)
QOF = [0, 512, 960]
NQC = len(QCS)
CIN = 256
H = 8
KD = 16
DV = 32
HKV = KD + DV            # 48 per-head kv channels
KVP = 64                 # padded kv rows: k at 0:16, v at 32:64
OC = 64                  # per-core slice of the 512 output channels
GRP = 2                  # k-tiles per exp group
NGRP = KT // GRP         # 21
EPS = 1e-5
SCALE = KD ** -0.5
NCORES = 8
XCS = [1344, 1344, 1344, 1344]       # x DMA chunks
XOF = [0, 1344, 2688, 4032]

LAST_EXEC_NS = None
_prog_cache = {}


def _build_program(debug=False):
    nc = bacc.Bacc(num_devices=NCORES)

    xT = nc.dram_tensor("xT", [B, 2, 128, NPAD], BF16, kind="ExternalInput")
    xsT = nc.dram_tensor("xsT", [B, 2, 128, NQ], BF16, kind="ExternalInput")
    wkqT = nc.dram_tensor("wkqT", [2, 128, KVP + KD], BF16,
                          kind="ExternalInput")
    wpT = nc.dram_tensor("wpT", [DV, 4, 128], BF16, kind="ExternalInput")
    gbT = nc.dram_tensor("gbT", [KVP, 6], F32, kind="ExternalInput")
    idT = nc.dram_tensor("idT", [KVP, DV], BF16, kind="ExternalInput")
    ebT = nc.dram_tensor("ebT", [NGRP, 128, GRP, NQ], BF16,
                         kind="ExternalInput")
    yT = nc.dram_tensor("yT", [OC, B * NQ], F32, kind="ExternalOutput")
    if debug:
        dbg = {
            "dbg_ykv": nc.dram_tensor("dbg_ykv", [KVP, B, NPAD], BF16,
                                      kind="ExternalOutput"),
            "dbg_qT": nc.dram_tensor("dbg_qT", [KD, B, NQ], BF16,
                                     kind="ExternalOutput"),
            "dbg_vaug": nc.dram_tensor("dbg_vaug", [128, B, KT, DV + 1], BF16,
                                       kind="ExternalOutput"),
            "dbg_hsT": nc.dram_tensor("dbg_hsT", [DV, B, NQ], BF16,
                                      kind="ExternalOutput"),
            "dbg_mvkv": nc.dram_tensor("dbg_mvkv", [KVP, 2], F32,
                                       kind="ExternalOutput"),
            "dbg_yfin": nc.dram_tensor("dbg_yfin", [OC, B, NQ], BF16,
                                       kind="ExternalOutput"),
        }

    with ExitStack() as ctx:
        tc = ctx.enter_context(tile.TileContext(nc))
        const = ctx.enter_context(tc.tile_pool(name="const", bufs=1))
        big = ctx.enter_context(tc.tile_pool(name="big", bufs=1))
        spool = ctx.enter_context(tc.tile_pool(name="spool", bufs=8))
        ebpool = ctx.enter_context(tc.tile_pool(name="ebpool", bufs=10))
        small = ctx.enter_context(tc.tile_pool(name="small", bufs=4))
        drain = ctx.enter_context(tc.tile_pool(name="drain", bufs=3))
        psA = ctx.enter_context(tc.tile_pool(name="psA", bufs=2, space="PSUM"))
        psB = ctx.enter_context(tc.tile_pool(name="psB", bufs=2, space="PSUM"))
        psC = ctx.enter_context(tc.tile_pool(name="psC", bufs=2, space="PSUM"))
        dram = ctx.enter_context(tc.tile_pool(name="dram", bufs=4, space="DRAM"))

        mult = mybir.AluOpType.mult
        add = mybir.AluOpType.add
        amin = mybir.AluOpType.min
        amax = mybir.AluOpType.max
        Act = mybir.ActivationFunctionType

        wkq_sb = const.tile([128, 2, KVP + KD], BF16, tag="wkq")
        wp_sb = const.tile([DV, 4, 128], BF16, tag="wp")
        gb_sb = const.tile([KVP, 6], F32, tag="gb")
        id_sb = const.tile([KVP, DV], BF16, tag="id")
        eps_t = const.tile([128, 1], F32, tag="eps")
        nc.vector.memset(eps_t, EPS)
        ones1_t = const.tile([1, DV], F32, tag="ones1")
        nc.vector.memset(ones1_t, 1.0)

        # v_aug gets its ones column once; the raw-v transposes fill 0:DV.
        v_aug = big.tile([128, B, KT, DV + 1], BF16, tag="vaug")
        nc.gpsimd.memset(v_aug[:, :, :, DV:DV + 1], 1.0)

        xt_sb = big.tile([128, B, 2, NPAD], BF16, tag="xt")
        xs_sb = big.tile([128, B, 2, NQ], BF16, tag="xs")
        y_kv = big.tile([KVP, B, NPAD], BF16, tag="ykv")
        y_q = big.tile([KD, B, NQ], BF16, tag="yq")
        st_kv = small.tile([KVP, 24, 6], F32, tag="st_kv")
        st_q = small.tile([KD, 6, 6], F32, tag="st_q")

        # ------------- pipelined x DMA + kv projection + stats -------------
        def kv_chunk(ch):
            X0, XW = XOF[ch], XCS[ch]
            for b in range(B):
                for c in range(2):
                    nc.sync.dma_start(
                        out=xt_sb[:, b, c, bass.ds(X0, XW)],
                        in_=xT[b, c, :, bass.ds(X0, XW)])
            if ch == 0:
                # weights land between chunk0 and chunk1 transfers
                for c in range(2):
                    nc.sync.dma_start(out=wkq_sb[:, c, :], in_=wkqT[c])
                nc.sync.dma_start(out=wp_sb, in_=wpT[:, :, :])
                nc.sync.dma_start(out=gb_sb, in_=gbT[:, :])
                nc.sync.dma_start(out=id_sb, in_=idT[:, :])
            for b in range(B):
                for u in range(XW // 448):
                    t = (X0 // 448) + u
                    ps = psB.tile([KVP, 448], F32, tag="ps_av")
                    for c in range(2):
                        nc.tensor.matmul(ps, wkq_sb[:, c, 0:KVP],
                                         xt_sb[:, b, c, bass.ds(t * 448, 448)],
                                         start=(c == 0), stop=(c == 1))
                    nc.scalar.copy(out=y_kv[:, b, bass.ds(t * 448, 448)],
                                   in_=ps)
                    # stats off the drained y_kv so the PSUM pipeline is
                    # drain-rate-limited (pads excluded via window width)
                    w = min(448, N - t * 448)
                    nc.vector.bn_stats(out=st_kv[:, t * B + b, :],
                                       in_=y_kv[:, b, bass.ds(t * 448, w)])

        kv_chunk(0)
        kv_chunk(1)
        # xs lands while kv chunk 2 streams
        for b in range(B):
            for c in range(2):
                nc.sync.dma_start(out=xs_sb[:, b, c, :], in_=xsT[b, c])
        kv_chunk(2)
        # q projection slots in while kv chunks 3/4 stream
        for b in range(B):
            for t in range(3):
                ps = psB.tile([KD, 448], F32, tag="ps_av")
                for c in range(2):
                    nc.tensor.matmul(ps, wkq_sb[:, c, KVP:KVP + KD],
                                     xs_sb[:, b, c, bass.ds(t * 448, 448)],
                                     start=(c == 0), stop=(c == 1))
                nc.scalar.copy(out=y_q[:, b, bass.ds(t * 448, 448)], in_=ps)
            for t in range(3):
                nc.vector.bn_stats(out=st_q[:, b * 3 + t, :],
                                   in_=y_q[:, b, bass.ds(t * 448, 448)])
        kv_chunk(3)

        # ------------- batch-norm scale/shift + q~ -------------
        def bn_scale_shift(mv, g_ap, b_ap, P, name):
            s = small.tile([P, 1], F32, tag=f"s_{name}")
            t = small.tile([P, 1], F32, tag=f"t_{name}")
            nc.scalar.activation(out=s, in_=mv[:, 1:2], func=Act.Sqrt,
                                 bias=eps_t[0:P])
            nc.vector.reciprocal(out=s, in_=s)
            nc.vector.tensor_mul(s, s, g_ap)
            nc.vector.tensor_mul(t, mv[:, 0:1], s)
            nc.vector.tensor_scalar(out=t, in0=t, scalar1=-1.0, scalar2=None,
                                    op0=mult)
            nc.vector.tensor_add(t, t, b_ap)
            return s, t

        mv_kv = small.tile([KVP, 2], F32, tag="mv_kv")
        nc.vector.bn_aggr(out=mv_kv, in_=st_kv)
        s_kv, t_kv = bn_scale_shift(mv_kv, gb_sb[:, 0:1], gb_sb[:, 1:2],
                                    KVP, "kv")
        mv_q = small.tile([KD, 2], F32, tag="mv_q")
        nc.vector.bn_aggr(out=mv_q, in_=st_q)
        s_q, t_q = bn_scale_shift(mv_q, gb_sb[0:KD, 2:3], gb_sb[0:KD, 3:4],
                                  KD, "q")

        # q~ = (s_k*s_q) . q_raw + s_k*t_q
        a_q = small.tile([KD, 1], F32, tag="a_q")
        b_q = small.tile([KD, 1], F32, tag="b_q")
        nc.vector.tensor_mul(a_q, s_kv[0:KD], s_q)
        nc.vector.tensor_mul(b_q, s_kv[0:KD], t_q)
        qT = big.tile([KD, B, NQ], BF16, tag="qT")
        for b in range(B):
            nc.vector.tensor_scalar(out=qT[:, b, :], in0=y_q[:, b, :],
                                    scalar1=a_q, scalar2=b_q,
                                    op0=mult, op1=add)
        # v-side scale/shift moved to base partition 0 for the drain ops
        s_v0 = small.tile([DV, 1], F32, tag="s_v0")
        t_v0 = small.tile([DV, 1], F32, tag="t_v0")
        nc.gpsimd.dma_start(out=s_v0, in_=s_kv[32:KVP])
        nc.gpsimd.dma_start(out=t_v0, in_=t_kv[32:KVP])
        c3_v = small.tile([DV, 1], F32, tag="c3v")
        nc.vector.tensor_scalar(out=c3_v, in0=t_v0, scalar1=3.0,
                                scalar2=None, op0=add)
        # raw v -> token-major via PE transposes (the DMA transpose engine
        # serializes against later DMAs in the tile scheduler, so avoid it)
        for b in range(B):
            for t0 in range(0, KT, 32):
                nt = min(32, KT - t0)
                pc_t = psC.tile([128, 1024], BF16, tag="pc")
                for i in range(nt):
                    nc.tensor.transpose(
                        pc_t[:, bass.ts(i, DV)],
                        y_kv[32:KVP, b, bass.ds((t0 + i) * 128, 128)],
                        id_sb[32:KVP, :])
                nc.vector.tensor_copy(v_aug[:, b, t0:t0 + nt, 0:DV],
                                      pc_t[:, 0:nt * DV])

        # ------------- attention + partial projection -------------
        hsT = big.tile([DV, B, NQ], BF16, tag="hsT")
        y_fin = big.tile([OC, B, NQ], BF16, tag="y_fin")
        y_out = big.tile([OC, B, NQ], F32, tag="y_out")
        y_bncs = []
        y_sls = []
        for i in range(NQC):
            ybnc_t = dram.tile([4, 128, B * QCS[i]], BF16, tag=f"bnc{i}")
            y_bncs.append(ybnc_t)
            ysl_t = dram.tile([OC, B * QCS[i]], BF16, tag=f"ysl{i}")
            y_sls.append(ysl_t)

        ypars = []
        for i in range(NQC):
            ypar_t = big.tile([128, 4, B, QCS[i]], BF16, tag=f"ypar{i}")
            ypars.append(ypar_t)

        def emit_proj_piece(qc, o, tail=False):
            # one <=512-wide partial-projection matmul per (b, out-block)
            W = QCS[qc]
            q0 = QOF[qc]
            for b in range(B):
                pc = psC.tile([128, 512], F32, tag="pc")
                nc.tensor.matmul(pc[:, 0:W], wp_sb[:, o, :],
                                 hsT[:, b, bass.ds(q0, W)],
                                 start=True, stop=True)
                dst = ypars[qc][:, o, b, 0:W]
                if tail and b == 0:
                    nc.scalar.copy(out=dst, in_=pc[:, 0:W])
                else:
                    nc.vector.tensor_copy(dst, pc[:, 0:W])

        def emit_rs(qc, tail=False):
            dma = nc.scalar.dma_start if tail else nc.gpsimd.dma_start
            dma(out=y_bncs[qc].rearrange("o p (b q) -> p o b q", b=B),
                in_=ypars[qc])
            nc.gpsimd.collective_compute(
                "ReduceScatter", add,
                replica_groups=[list(range(NCORES))],
                ins=[y_bncs[qc].opt()],
                outs=[y_sls[qc].opt()])

        def attn_group(g, b, av_t, eb, W, q0):
            qk = psA.tile([128, GRP, 512], F32, tag="qk")
            for i in range(GRP):
                j = g * GRP + i
                nc.tensor.matmul(qk[:, i, 0:W],
                                 y_kv[0:KD, b, bass.ts(j, 128)],
                                 qT[:, b, bass.ds(q0, W)],
                                 start=True, stop=True)
            sp = spool.tile([128, GRP, 512], BF16, tag="sp")
            nc.scalar.activation(out=sp[:, :, 0:W], in_=qk[:, :, 0:W],
                                 func=Act.Exp, scale=SCALE)
            nc.vector.tensor_mul(sp[:, :, 0:W], sp[:, :, 0:W], eb[:, :, 0:W])
            for i in range(GRP):
                j = g * GRP + i
                nc.tensor.matmul(av_t[:, 0:W], v_aug[:, b, j, :],
                                 sp[:, i, 0:W],
                                 start=(j == 0), stop=(j == KT - 1),
                                 skip_group_check=True)

        def drain_chain(b, av_t, W, q0, alt):
            # out = (av*s_v)/denom + t_v, then hardswish.  alt=True parks the
            # accumulator via ACT (tail: both batches drain in parallel).
            av = drain.tile([DV + 1, 512], F32, tag="av_sb")
            if alt:
                nc.scalar.copy(out=av[:, 0:W], in_=av_t[:, 0:W])
            else:
                nc.vector.tensor_copy(av[:, 0:W], av_t[:, 0:W])
            rec = drain.tile([1, 512], F32, tag="rec")
            nc.vector.reciprocal(out=rec[:, 0:W], in_=av[DV:DV + 1, 0:W])
            pc_r = psC.tile([128, 512], F32, tag="pc")
            recb = pc_r[0:DV, :]
            nc.tensor.matmul(recb[:, 0:W], ones1_t, rec[:, 0:W],
                             start=True, stop=True)
            xo = drain.tile([DV, 512], F32, tag="xo")
            nc.vector.scalar_tensor_tensor(
                out=xo[:, 0:W], in0=av[0:DV, 0:W], scalar=s_v0,
                in1=recb[:, 0:W], op0=mult, op1=mult)
            r3 = drain.tile([DV, 512], F32, tag="r3")
            nc.vector.tensor_scalar(out=r3[:, 0:W], in0=xo[:, 0:W],
                                    scalar1=c3_v, scalar2=0.0,
                                    op0=add, op1=amax)
            nc.vector.tensor_scalar(out=r3[:, 0:W], in0=r3[:, 0:W],
                                    scalar1=6.0, scalar2=1.0 / 6.0,
                                    op0=amin, op1=mult)
            nc.vector.scalar_tensor_tensor(
                out=hsT[:, b, bass.ds(q0, W)], in0=xo[:, 0:W],
                scalar=t_v0, in1=r3[:, 0:W], op0=add, op1=mult)

        for qc in range(NQC - 1):
            W = QCS[qc]
            q0 = QOF[qc]
            avs = []
            for _b in range(B):
                av_t = psB.tile([DV + 1, 512], F32, tag="ps_av")
                avs.append(av_t)
            for g in range(NGRP):
                eb = ebpool.tile([128, GRP, 512], BF16, tag="eb")
                nc.sync.dma_start(out=eb[:, :, 0:W],
                                  in_=ebT[g, :, :, bass.ds(q0, W)])
                if qc > 0 and 5 <= g < 13 and g % 2 == 1:
                    emit_proj_piece(qc - 1, (g - 5) // 2)
                if qc > 0 and g == 13:
                    emit_rs(qc - 1)
                for b in range(B):
                    attn_group(g, b, avs[b], eb, W, q0)
            for b in range(B):
                drain_chain(b, avs[b], W, q0, False)
        # ---- tail chunk, b-outer: b0's drain/proj/bounce hide under b1 ----
        qc = NQC - 1
        W = QCS[qc]
        q0 = QOF[qc]
        for b in range(B):
            av_t = psB.tile([DV + 1, 512], F32, tag="ps_av")
            for g in range(NGRP):
                eb = ebpool.tile([128, GRP, 512], BF16, tag="eb")
                nc.sync.dma_start(out=eb[:, :, 0:W],
                                  in_=ebT[g, :, :, bass.ds(q0, W)])
                if b == 0 and 5 <= g < 13 and g % 2 == 1:
                    emit_proj_piece(qc - 1, (g - 5) // 2)
                if b == 0 and g == 13:
                    emit_rs(qc - 1)
                attn_group(g, b, av_t, eb, W, q0)
            drain_chain(b, av_t, W, q0, alt=(b == 1))
            for o in range(4):
                pc = psC.tile([128, 512], F32, tag="pc")
                nc.tensor.matmul(pc[:, 0:W], wp_sb[:, o, :],
                                 hsT[:, b, bass.ds(q0, W)],
                                 start=True, stop=True)
                dst = ypars[qc][:, o, b, 0:W]
                if b == 0:
                    nc.vector.tensor_copy(dst, pc[:, 0:W])
                else:
                    nc.scalar.copy(out=dst, in_=pc[:, 0:W])
            half_dma = nc.gpsimd.dma_start
            half_dma(
                out=y_bncs[qc][:, :, bass.ds(b * W, W)].rearrange(
                    "o p q -> p o q"),
                in_=ypars[qc][:, :, b, :])
        nc.gpsimd.collective_compute(
            "ReduceScatter", add,
            replica_groups=[list(range(NCORES))],
            ins=[y_bncs[qc].opt()],
            outs=[y_sls[qc].opt()])
        # preload the Sqrt activation table while the collective runs
        sq_warm = small.tile([1, 1], F32, tag="sq_warm")
        nc.scalar.activation(out=sq_warm, in_=eps_t[0:1, :], func=Act.Sqrt,
                             bias=eps_t[0:1])

        if debug:
            nc.sync.dma_start(out=dbg["dbg_ykv"][:, :, :], in_=y_kv)
            nc.sync.dma_start(out=dbg["dbg_qT"][:, :, :], in_=qT)
            nc.sync.dma_start(out=dbg["dbg_vaug"][:, :, :, :], in_=v_aug)
            nc.sync.dma_start(out=dbg["dbg_hsT"][:, :, :], in_=hsT)
            nc.sync.dma_start(out=dbg["dbg_mvkv"][:, :], in_=mv_kv)

        # ------------- gather slices + final BN (kept off queue heads) -----
        ctx.enter_context(tc.tile_wait_until(0.25))
        st_p = small.tile([OC, B * NQC, 6], F32, tag="st_p")
        for qc in range(NQC):
            W = QCS[qc]
            for b in range(B):
                nc.sync.dma_start(
                    out=y_fin[:, b, bass.ds(QOF[qc], W)],
                    in_=y_sls[qc][:, bass.ds(b * W, W)])
                nc.vector.bn_stats(out=st_p[:, qc * B + b, :],
                                   in_=y_fin[:, b, bass.ds(QOF[qc], W)])
        mv_p = small.tile([OC, 2], F32, tag="mv_p")
        nc.vector.bn_aggr(out=mv_p, in_=st_p)
        s_p, t_p = bn_scale_shift(mv_p, gb_sb[:, 4:5], gb_sb[:, 5:6],
                                  OC, "p")
        nc.vector.tensor_scalar(out=y_out[:, 0, :], in0=y_fin[:, 0, :],
                                scalar1=s_p, scalar2=t_p, op0=mult, op1=add)
        nc.vector.tensor_scalar(out=y_out[:, 1, :], in0=y_fin[:, 1, :],
                                scalar1=s_p, scalar2=t_p, op0=mult, op1=add)
        if debug:
            nc.sync.dma_start(out=dbg["dbg_yfin"][:, :, :], in_=y_fin)
        nc.sync.dma_start(out=yT[:, 0:NQ], in_=y_out[:, 0, :])
        nc.sync.dma_start(out=yT[:, NQ:2 * NQ], in_=y_out[:, 1, :])

    nc.finalize()
    return nc


def _prep_inputs(x, kv_w, kv_g, kv_b, q_w, q_g, q_b, proj_w, proj_g, proj_b,
                 bias_table, bias_idxs):
    """Host-side sharding/layout prep. Returns list of 8 per-core input maps."""
    x = np.asarray(x, np.float32)
    xt = np.zeros((B, 2, 128, NPAD), np.float32)
    xTt = x.transpose(0, 2, 1)  # (B, 256, N)
    xt[:, :, :, :N] = xTt.reshape(B, 2, 128, N)
    xt = xt.astype(bf16)

    # exp(bias) tables per head, padded-k zeroed, laid out (NGRP, 128, GRP, NQ)
    rank2 = np.asarray(bias_idxs)[0].reshape(ROW, COL)  # (dr, dc) -> id
    table2 = np.asarray(bias_table, np.float32)[:, rank2]  # (H, 63, 84)
    eb2 = np.exp(table2)
    kk = np.arange(N)
    qq = np.arange(NQ)
    DRm = np.abs(kk[:, None] // COL - 2 * (qq[None, :] // COL_))
    DCm = np.abs(kk[:, None] % COL - 2 * (qq[None, :] % COL_))

    xs = x.reshape(B, ROW, COL, CIN)[:, ::2, ::2].reshape(B, NQ, CIN)
    xst = xs.transpose(0, 2, 1).reshape(B, 2, 128, NQ).astype(bf16)
    in_maps = []
    for h in range(H):
        ebf = np.zeros((NPAD, NQ), np.float32)
        ebf[:N] = eb2[h][DRm, DCm]
        ebl = np.ascontiguousarray(
            ebf.reshape(NGRP, GRP, 128, NQ).transpose(0, 2, 1, 3)
        ).astype(bf16)
        sl = slice(h * HKV, (h + 1) * HKV)
        slq = slice(h * KD, (h + 1) * KD)
        slo = slice(h * OC, (h + 1) * OC)
        wkv_pad = np.zeros((KVP, CIN), np.float32)
        wkv_pad[0:KD] = np.asarray(kv_w, np.float32)[sl][0:KD]
        wkv_pad[32:KVP] = np.asarray(kv_w, np.float32)[sl][KD:HKV]
        # combined kv+q weights: (2, 128, KVP+KD)
        wkq = np.zeros((CIN, KVP + KD), np.float32)
        wkq[:, 0:KVP] = wkv_pad.T
        wkq[:, KVP:] = np.asarray(q_w, np.float32)[slq].T
        # packed gains/biases: kv at [:,0:2], q at rows 0:16 [:,2:4],
        # proj at [:,4:6]
        gb = np.zeros((KVP, 6), np.float32)
        gb[:, 0] = 1.0
        gb[0:KD, 0] = np.asarray(kv_g, np.float32)[sl][0:KD]
        gb[0:KD, 1] = np.asarray(kv_b, np.float32)[sl][0:KD]
        gb[32:KVP, 0] = np.asarray(kv_g, np.float32)[sl][KD:HKV]
        gb[32:KVP, 1] = np.asarray(kv_b, np.float32)[sl][KD:HKV]
        gb[0:KD, 2] = np.asarray(q_g, np.float32)[slq]
        gb[0:KD, 3] = np.asarray(q_b, np.float32)[slq]
        gb[:, 4] = np.asarray(proj_g, np.float32)[slo]
        gb[:, 5] = np.asarray(proj_b, np.float32)[slo]
        # W_p columns for this head's 32 v-channels, as 4 stationary tiles
        wp_h = np.asarray(proj_w, np.float32)[:, h * DV:(h + 1) * DV]
        wp_t = np.ascontiguousarray(wp_h.T.reshape(DV, 4, 128)).astype(bf16)
        in_maps.append({
            "xT": xt,
            "xsT": xst,
            "idT": np.concatenate([np.zeros((DV, DV), np.float32), np.eye(DV, dtype=np.float32)]).astype(bf16),
            "wkqT": np.ascontiguousarray(
                wkq.reshape(2, 128, KVP + KD)).astype(bf16),
            "wpT": wp_t,
            "gbT": np.ascontiguousarray(gb),
            "ebT": ebl,
        })
    return in_maps


def kernel(x, kv_w, kv_g, kv_b, q_w, q_g, q_b, proj_w, proj_g, proj_b,
           bias_table, bias_idxs, _trace=False):
    global LAST_EXEC_NS
    if "nc" not in _prog_cache:
        _prog_cache["nc"] = _build_program()
    nc = _prog_cache["nc"]
    in_maps = _prep_inputs(x, kv_w, kv_g, kv_b, q_w, q_g, q_b,
                           proj_w, proj_g, proj_b, bias_table, bias_idxs)
    res = run_bass_kernel_spmd(nc, in_maps, core_ids=list(range(NCORES)),
                               trace=_trace)
    LAST_EXEC_NS = res.exec_time_ns
    yts = [np.asarray(r["yT"]) for r in res.results]  # each (OC, B*NQ)
    y = np.concatenate(yts, axis=0)                   # (512, B*NQ)
    return np.ascontiguousarray(
        y.T.reshape(B, NQ, H * OC).astype(np.float32))
